# revision 18
# baseline (speedup 1.0000x reference)
"""Trainium2 Bass kernel for nn_DependencyNeuralModel (dependency parser scorer).

Device strategy (8 NeuronCores, SPMD):
  - Encoder (2-layer BiLSTM over S=512) replicated on every core,
    chunk-parallel: 64 chunks x 2 dirs = 128 batch rows advance in
    lock-step; each chunk warms up from zero over K=16 positions
    (forget-gate contraction makes the warmup error ~6e-3).
    Per step the wx term is injected into PSUM by an identity matmul and
    the gate nonlinearities read PSUM directly, one 512-wide gate region
    at a time, so ACT overlaps the PE stream.
  - Arc scores: A = S^2 exactly, so the full score matrix
    Score[h,m] = w . tanh(headsT[:,h] + modsT[:,m] + DT[:, m-h]) is
    computed densely with NO gather: partition dim = hidden (4 chunks),
    mods as a resident tile, heads column as the ACT bias, and the
    distance embedding as a sliding window into an offset table.
    h rows are sharded across cores (64 per core); the host does the
    final Score[arc_head, arc_mod] scalar fancy-index.
  - Sibling scores: parts bucket-sorted on host by
    (head//128, mod//128, sib//128) into 64 buckets x 3 static tiles;
    each tile needs just 3 one-hot gather matmuls (one 128-row chunk per
    role).  tanh on ACT, fused multiply+reduce on DVE.
Host does index/layout preparation, the final arc fancy-index and
sibling unpermute.
"""
import sys
import types

import numpy as np

sys.path.insert(0, "/opt/trn_rl_repo")

import concourse.bass as bass
import concourse.mybir as mybir
from concourse.tile import TileContext
from concourse.masks import make_identity

S = 512
H = 512
A = 262144
ASIB = 131072
NB = 17
L = 8
K_WARM = 16
NSTEP = K_WARM + L  # 24
NC = 8
F32 = mybir.dt.float32
BF16 = mybir.dt.bfloat16
BINS = np.array(list(range(10)) + list(range(10, 40, 5)) + [40], dtype=np.int64)

N_BUCKET = 64          # (head//128, mod//128, sib//128)
TILES_PER_BUCKET = 3   # 384 slots per bucket; max observed occupancy ~306
N_SIB_TILE = N_BUCKET * TILES_PER_BUCKET  # 192
N_ARC_H = S // NC      # 64 dense score-matrix rows per core
MASK_STEPS = {7: 0, 15: 1}


def _install_ntff_hook():
    if "antenv.axon_hooks" in sys.modules:
        return
    mod = types.ModuleType("antenv.axon_hooks")
    state = {"hook": None, "tried": False}

    def set_axon_ntff_profile_hook(hook):
        state["hook"] = hook

    def get_axon_ntff_profile_hook():
        if state["hook"] is None and not state["tried"]:
            state["tried"] = True
            try:
                from trn_agent_boot.trn_boot import _ntff_profile_via_ctypes

                state["hook"] = _ntff_profile_via_ctypes("/opt/axon/libaxon_pjrt.so")
            except Exception:
                state["hook"] = None
        return state["hook"]

    mod.set_axon_ntff_profile_hook = set_axon_ntff_profile_hook
    mod.get_axon_ntff_profile_hook = get_axon_ntff_profile_hook
    import antenv

    antenv.axon_hooks = mod
    sys.modules["antenv.axon_hooks"] = mod


def _legalize_waits(nc):
    """This walrus accepts at most one semaphore wait per instruction;
    split extra waits onto same-engine NOPs placed just before."""
    ctr = [0]
    for f in nc.m.functions:
        for blk in f.blocks:
            out = []
            dirty = False
            for ins in blk.instructions:
                si = ins.sync_info
                if si is not None and si.on_wait and len(si.on_wait) > 1:
                    waits = list(si.on_wait)
                    for w in waits[:-1]:
                        ctr[0] += 1
                        nop = mybir.InstNoOp(name=f"waitfix-{ctr[0]}")
                        nop.engine = ins.engine
                        nop.sync_info = mybir.SyncInfo(on_wait=[w], on_update=[])
                        out.append(nop)
                    ins.sync_info = mybir.SyncInfo(
                        on_wait=[waits[-1]],
                        on_update=list(si.on_update) if si.on_update else [],
                    )
                    dirty = True
                out.append(ins)
            if dirty:
                blk.instructions = out
    return nc


def _lstm_layer(nc, tc, ident, mask_sb, whh_sb, wx_dram, f_dram, b_dram):
    """One BiLSTM layer, chunk-parallel.  128 batch rows: partitions 0:64
    dir0 chunks, 64:128 dir1.  Gate regions (512 cols each) get their own
    PSUM tile; wx is injected by an identity matmul so ACT reads PSUM."""
    import contextlib

    SIG = mybir.ActivationFunctionType.Sigmoid
    TANH = mybir.ActivationFunctionType.Tanh

    with contextlib.ExitStack() as ctx:
        wxp = ctx.enter_context(tc.tile_pool(name="lstm_wx", bufs=2))
        pg = ctx.enter_context(tc.tile_pool(name="lstm_pg", bufs=6, space="PSUM"))
        gp = ctx.enter_context(tc.tile_pool(name="lstm_g", bufs=6))
        cp = ctx.enter_context(tc.tile_pool(name="lstm_c", bufs=8))
        pst = ctx.enter_context(tc.tile_pool(name="lstm_pst", bufs=2, space="PSUM"))
        st = ctx.enter_context(tc.tile_pool(name="lstm_state", bufs=1))

        h_t = st.tile([128, 4, 128], BF16)  # h transposed: [k-part, kc, b]
        c_st = st.tile([128, 512], F32)
        nc.vector.memset(h_t.rearrange("p a b -> p (a b)"), 0.0)
        nc.vector.memset(c_st[:], 0.0)

        for s in range(NSTEP):
            wx = wxp.tile([128, 2048], BF16, tag="wx")
            for d in range(2):
                nc.sync.dma_start(
                    wx[d * 64:(d + 1) * 64, :], wx_dram[d, s:s + 505:8, :]
                )
            # all four wx-injection matmuls first: they depend only on wx, so
            # they fill the PE gap while the previous step's tail completes
            P = {}
            for ng in (1, 0, 2, 3):  # f, i, g, o
                Pt = pg.tile([128, 512], F32, tag="P")
                P[ng] = Pt
                nc.tensor.matmul(
                    Pt[:], lhsT=ident[:], rhs=wx[:, ng * 512:(ng + 1) * 512],
                    start=True, stop=False, skip_group_check=True,
                )
            gate = {}
            for ng in (1, 0, 2, 3):
                for d in range(2):
                    bs = slice(d * 64, (d + 1) * 64)
                    for kc in range(4):
                        nc.tensor.matmul(
                            P[ng][bs, :],
                            lhsT=h_t[:, kc, bs],
                            rhs=whh_sb[:, kc, d, ng * 512:(ng + 1) * 512],
                            start=False,
                            stop=(d == 1 and kc == 3),
                            skip_group_check=True,
                        )
                g = gp.tile([128, 512], BF16, tag=f"g{ng}")
                nc.scalar.activation(g[:], P[ng][:], TANH if ng == 2 else SIG)
                gate[ng] = g
            t1 = cp.tile([128, 512], F32, tag="t1")
            nc.vector.tensor_mul(t1[:], gate[1][:], c_st[:])
            t2 = cp.tile([128, 512], F32, tag="t2")
            nc.vector.tensor_mul(t2[:], gate[0][:], gate[2][:])
            nc.vector.tensor_add(c_st[:], t1[:], t2[:])
            # tail in halves so transposes/copies overlap the second tanh
            h_new = cp.tile([128, 512], BF16, tag="h")
            mi = MASK_STEPS.get(s)
            for half in range(2):
                sl = slice(half * 256, (half + 1) * 256)
                tch = cp.tile([128, 256], BF16, tag=f"tch{half}")
                nc.scalar.activation(tch[:], c_st[:, sl], TANH)
                nc.vector.tensor_mul(h_new[:, sl], gate[3][:, sl], tch[:])
                if mi is not None:
                    nc.vector.tensor_scalar_mul(h_new[:, sl], h_new[:, sl],
                                                mask_sb[:, mi:mi + 1])
                for kc in (2 * half, 2 * half + 1):
                    tp = pst.tile([128, 128], BF16, tag="tr")
                    nc.tensor.transpose(tp[:], h_new[:, kc * 128:(kc + 1) * 128],
                                        ident[:])
                    nc.vector.tensor_copy(h_t[:, kc, :], tp[:])
            if mi is not None:
                nc.vector.tensor_scalar_mul(c_st[:], c_st[:], mask_sb[:, mi:mi + 1])
            if s >= K_WARM:
                o = s - K_WARM
                nc.sync.dma_start(f_dram[o:505 + o:8, :], h_new[0:64, :])
                nc.sync.dma_start(b_dram[o:505 + o:8, :], h_new[64:128, :])


def _transpose_pair(nc, tc, ident, rev, f_dram, b_dram, dstT, dstTrev, one_row):
    """Build [feat, pos] lhsT chunks (and optionally pos-reversed copy) from
    the per-direction output buffers.  dstT/dstTrev: [128, 9, 512] tiles;
    chunk 8 row 0 is set to ones (bias); rest of chunk 8 zero."""
    import contextlib

    with contextlib.ExitStack() as ctx:
        sb = ctx.enter_context(tc.tile_pool(name="tp_sb", bufs=3))
        ps = ctx.enter_context(tc.tile_pool(name="tp_ps", bufs=2, space="PSUM"))
        for dst in (dstT, dstTrev):
            if dst is None:
                continue
            nc.vector.memset(dst[:, 8, :], 0.0)
            nc.vector.tensor_copy(dst[0:1, 8, :], one_row[:])
        for pc in range(4):
            fsrc = sb.tile([128, 512], BF16, tag="fsrc")
            nc.sync.dma_start(fsrc[:], f_dram[pc * 128:(pc + 1) * 128, :])
            bsrc = sb.tile([128, 512], BF16, tag="bsrc")
            nc.sync.dma_start(bsrc[:], b_dram[pc * 128:(pc + 1) * 128, :])
            for j in range(4):
                fs = fsrc[:, j * 128:(j + 1) * 128]
                bs = bsrc[:, j * 128:(j + 1) * 128]
                tp = ps.tile([128, 128], BF16, tag="tp")
                nc.tensor.transpose(tp[:], fs, ident[:])
                nc.vector.tensor_copy(dstT[:, j, pc * 128:(pc + 1) * 128], tp[:])
                if dstTrev is not None:
                    tpr = ps.tile([128, 128], BF16, tag="tpr")
                    nc.tensor.transpose(tpr[:], fs, rev[:])
                    nc.vector.tensor_copy(
                        dstTrev[:, j, (3 - pc) * 128:(4 - pc) * 128], tpr[:])
                # b rows are scan order q; position = 511-q: reverse via rev
                tpb = ps.tile([128, 128], BF16, tag="tpb")
                nc.tensor.transpose(tpb[:], bs, rev[:])
                nc.vector.tensor_copy(
                    dstT[:, 4 + j, (3 - pc) * 128:(4 - pc) * 128], tpb[:])
                if dstTrev is not None:
                    tpb2 = ps.tile([128, 128], BF16, tag="tpb2")
                    nc.tensor.transpose(tpb2[:], bs, ident[:])
                    nc.vector.tensor_copy(
                        dstTrev[:, 4 + j, pc * 128:(pc + 1) * 128], tpb2[:])


def _input_gemm(nc, tc, lhsT_tiles, w_sb, wx_dram, nk):
    """WX[d] = lhsT_d.T @ w[d] -> wx_dram[d, 16:528, :], bf16.
    lhsT_tiles: per-dir [128, nk, 512] SBUF ([feat-part, chunk, pos]).
    w_sb: [128, nk, 2, 2048] SBUF weights."""
    import contextlib

    with contextlib.ExitStack() as ctx:
        sb = ctx.enter_context(tc.tile_pool(name="ig_sb", bufs=4))
        ps = ctx.enter_context(tc.tile_pool(name="ig_ps", bufs=4, space="PSUM"))
        for d in range(2):
            lhsT = lhsT_tiles[d]
            for mc in range(4):
                for ngc in range(4):
                    acc = ps.tile([128, 512], F32, tag="acc")
                    for kc in range(nk):
                        nc.tensor.matmul(
                            acc[:],
                            lhsT=lhsT[:, kc, mc * 128:(mc + 1) * 128],
                            rhs=w_sb[:, kc, d, ngc * 512:(ngc + 1) * 512],
                            start=(kc == 0),
                            stop=(kc == nk - 1),
                        )
                    osb = sb.tile([128, 512], BF16, tag="osb")
                    if (mc + ngc) % 2 == 0:
                        nc.vector.tensor_copy(osb[:], acc[:])
                    else:
                        nc.scalar.copy(osb[:], acc[:])
                    nc.sync.dma_start(
                        wx_dram[d, 16 + mc * 128:16 + (mc + 1) * 128,
                                ngc * 512:(ngc + 1) * 512],
                        osb[:],
                    )


def _build(nc):
    TANH = mybir.ActivationFunctionType.Tanh
    embT_f = nc.dram_tensor("embT_f", [128, 3, 512], BF16, kind="ExternalInput")
    embT_b = nc.dram_tensor("embT_b", [128, 3, 512], BF16, kind="ExternalInput")
    wih0T = nc.dram_tensor("wih0T", [128, 3, 2, 2048], BF16, kind="ExternalInput")
    whh0T = nc.dram_tensor("whh0T", [128, 4, 2, 2048], BF16, kind="ExternalInput")
    wih1T = nc.dram_tensor("wih1T", [128, 9, 2, 2048], BF16, kind="ExternalInput")
    whh1T = nc.dram_tensor("whh1T", [128, 4, 2, 2048], BF16, kind="ExternalInput")
    projT = nc.dram_tensor("projT", [128, 9, 2560], BF16, kind="ExternalInput")
    dtexp = nc.dram_tensor("dtexp", [128, 4, 1023], BF16, kind="ExternalInput")
    selT = nc.dram_tensor("selT", [128, 4, 64], BF16, kind="ExternalInput")
    wT_in = nc.dram_tensor("wT_in", [128, 4], BF16, kind="ExternalInput")
    wrep_in = nc.dram_tensor("wrep_in", [128, 512], BF16, kind="ExternalInput")
    sibidx = nc.dram_tensor("sibidx", [N_SIB_TILE, 384], BF16,
                            kind="ExternalInput")
    iota_in = nc.dram_tensor("iota_in", [128, 1], F32, kind="ExternalInput")
    mask_in = nc.dram_tensor("mask_in", [128, 2], F32, kind="ExternalInput")
    rev_in = nc.dram_tensor("rev_in", [128, 128], BF16, kind="ExternalInput")
    arc_out = nc.dram_tensor("arc_out", [N_ARC_H, 512], F32, kind="ExternalOutput")
    sib_out = nc.dram_tensor("sib_out", [128, N_SIB_TILE], F32,
                             kind="ExternalOutput")

    wx0 = nc.dram_tensor("wx0", [2, 528, 2048], BF16)
    wx1 = nc.dram_tensor("wx1", [2, 528, 2048], BF16)
    f0d = nc.dram_tensor("f0d", [512, 512], BF16)
    b0d = nc.dram_tensor("b0d", [512, 512], BF16)
    f1d = nc.dram_tensor("f1d", [512, 512], BF16)
    b1d = nc.dram_tensor("b1d", [512, 512], BF16)

    import contextlib

    with TileContext(nc) as tc:
        with contextlib.ExitStack() as ctx:
            const = ctx.enter_context(tc.tile_pool(name="const", bufs=1))
            enc = ctx.enter_context(tc.tile_pool(name="enc", bufs=1))
            sco = ctx.enter_context(tc.tile_pool(name="sco", bufs=1))

            ident = const.tile([128, 128], BF16)
            make_identity(nc, ident[:])
            rev = const.tile([128, 128], BF16)
            nc.sync.dma_start(rev[:], rev_in[:])
            iota_sb = const.tile([128, 1], F32)
            nc.sync.dma_start(iota_sb[:], iota_in[:])
            mask_sb = const.tile([128, 2], F32)
            nc.sync.dma_start(mask_sb[:], mask_in[:])
            wT_sb = const.tile([128, 4], BF16)
            nc.sync.dma_start(wT_sb[:], wT_in[:])
            wrep = const.tile([128, 512], BF16)
            nc.sync.dma_start(wrep[:], wrep_in[:])
            sel_sb = const.tile([128, 4, 64], BF16)
            nc.sync.dma_start(sel_sb.rearrange("p a b -> p (a b)"),
                              selT.rearrange("p a b -> p (a b)"))
            one_row = const.tile([1, 512], BF16)
            nc.vector.memset(one_row[:], 1.0)

            # zero-pad warmup rows of WX buffers
            with tc.tile_pool(name="zp", bufs=1) as zp:
                zrow = zp.tile([16, 2048], BF16)
                nc.vector.memset(zrow[:], 0.0)
                for wxd in (wx0, wx1):
                    for d in range(2):
                        nc.sync.dma_start(wxd[d, 0:16, :], zrow[:])

            # ---- WX0 ----
            with tc.tile_pool(name="w0", bufs=1) as w0p:
                wih0_sb = w0p.tile([128, 3, 2, 2048], BF16)
                nc.sync.dma_start(wih0_sb.rearrange("p a b c -> p (a b c)"),
                                  wih0T.rearrange("p a b c -> p (a b c)"))
                ef = w0p.tile([128, 3, 512], BF16)
                nc.sync.dma_start(ef.rearrange("p a b -> p (a b)"),
                                  embT_f.rearrange("p a b -> p (a b)"))
                eb = w0p.tile([128, 3, 512], BF16)
                nc.sync.dma_start(eb.rearrange("p a b -> p (a b)"),
                                  embT_b.rearrange("p a b -> p (a b)"))
                _input_gemm(nc, tc, [ef, eb], wih0_sb, wx0, 3)

                # ---- layer 0 (whh0 shares this scope's lifetime) ----
                whh0_sb = w0p.tile([128, 4, 2, 2048], BF16)
                nc.sync.dma_start(whh0_sb.rearrange("p a b c -> p (a b c)"),
                                  whh0T.rearrange("p a b c -> p (a b c)"))
                _lstm_layer(nc, tc, ident, mask_sb, whh0_sb, wx0, f0d, b0d)

            # ---- x1T / x1Trev ----
            x1T = enc.tile([128, 9, 512], BF16, tag="x1T")
            x1Trev = enc.tile([128, 9, 512], BF16, tag="x1Trev")
            _transpose_pair(nc, tc, ident, rev, f0d, b0d, x1T, x1Trev, one_row)

            # ---- WX1 + layer 1 ----
            with tc.tile_pool(name="w1", bufs=1) as w1p:
                wih1_sb = w1p.tile([128, 9, 2, 2048], BF16)
                nc.sync.dma_start(wih1_sb.rearrange("p a b c -> p (a b c)"),
                                  wih1T.rearrange("p a b c -> p (a b c)"))
                _input_gemm(nc, tc, [x1T, x1Trev], wih1_sb, wx1, 9)
            with tc.tile_pool(name="w1b", bufs=1) as w1bp:
                whh1_sb = w1bp.tile([128, 4, 2, 2048], BF16)
                nc.sync.dma_start(whh1_sb.rearrange("p a b c -> p (a b c)"),
                                  whh1T.rearrange("p a b c -> p (a b c)"))
                _lstm_layer(nc, tc, ident, mask_sb, whh1_sb, wx1, f1d, b1d)

            # ---- statesT ----
            stT = enc.tile([128, 9, 512], BF16, tag="x1T")  # reuse slot
            _transpose_pair(nc, tc, ident, rev, f1d, b1d, stT, None, one_row)

            # ---- projection tables ----
            tables_sb = sco.tile([128, 4, 1536], BF16, tag="tables")
            heads_pos = sco.tile([128, 4, 512], BF16, tag="heads_pos")
            modsT = sco.tile([128, 4, 512], BF16, tag="modsT")
            headsb = sco.tile([128, 4, 64], F32, tag="headsb")
            with contextlib.ExitStack() as c2:
                pj = c2.enter_context(tc.tile_pool(name="pj", bufs=1))
                ps2 = c2.enter_context(tc.tile_pool(name="tb_ps", bufs=5,
                                                    space="PSUM"))
                ps2b = c2.enter_context(tc.tile_pool(name="tb_ps2", bufs=1,
                                                     space="PSUM"))
                ps2c = c2.enter_context(tc.tile_pool(name="tb_ps3", bufs=2,
                                                     space="PSUM"))
                projT_sb = pj.tile([128, 9, 2560], BF16)
                nc.sync.dma_start(projT_sb.rearrange("p a b -> p (a b)"),
                                  projT.rearrange("p a b -> p (a b)"))
                # sib tables + heads in pos-part layout
                for mc in range(4):
                    for r in range(4):  # 0..2 sib tables, 3 = heads
                        toff = (2 + r) * 512 if r < 3 else 0
                        acc = ps2.tile([128, 512], F32, tag="acc")
                        for kc in range(9):
                            nc.tensor.matmul(
                                acc[:],
                                lhsT=stT[:, kc, mc * 128:(mc + 1) * 128],
                                rhs=projT_sb[:, kc, toff:toff + 512],
                                start=(kc == 0), stop=(kc == 8),
                            )
                        if r < 3:
                            nc.vector.tensor_copy(
                                tables_sb[:, mc, r * 512:(r + 1) * 512], acc[:])
                        else:
                            nc.scalar.copy(heads_pos[:, mc, :], acc[:])
                # mods in hidden-part layout
                for hc in range(4):
                    acc = ps2.tile([128, 512], F32, tag="acc")
                    for kc in range(9):
                        nc.tensor.matmul(
                            acc[:],
                            lhsT=projT_sb[:, kc, 512 + hc * 128:512 + hc * 128 + 128],
                            rhs=stT[:, kc, :],
                            start=(kc == 0), stop=(kc == 8),
                        )
                    nc.vector.tensor_copy(modsT[:, hc, :], acc[:])
                # per-core heads columns: select 64 pos, then transpose
                hsel_ps = ps2b.tile([64, 512], F32, tag="hsel")
                for mc in range(4):
                    nc.tensor.matmul(
                        hsel_ps[:], lhsT=sel_sb[:, mc, :], rhs=heads_pos[:, mc, :],
                        start=(mc == 0), stop=(mc == 3),
                    )
                hsel_sb = pj.tile([64, 512], BF16)
                nc.scalar.copy(hsel_sb[:], hsel_ps[:])
                for hc in range(4):
                    tp = ps2c.tile([128, 64], BF16, tag="htp")
                    nc.tensor.transpose(tp[:], hsel_sb[:, hc * 128:(hc + 1) * 128],
                                        ident[0:64, 0:64])
                    nc.vector.tensor_copy(headsb[:, hc, :], tp[:])

            dtexp_sb = sco.tile([128, 4, 1023], BF16, tag="dtexp")
            nc.sync.dma_start(dtexp_sb.rearrange("p a b -> p (a b)"),
                              dtexp.rearrange("p a b -> p (a b)"))
            sib_sb = sco.tile([128, N_SIB_TILE], F32, tag="sib_sb")

            # ---- scoring: interleave sib tiles (PE/DVE) with arc rows (ACT) --
            with contextlib.ExitStack() as c3:
                ip = c3.enter_context(tc.tile_pool(name="sc_i", bufs=2))
                bcp = c3.enter_context(tc.tile_pool(name="sc_bc", bufs=2,
                                                    space="PSUM"))
                ohp = c3.enter_context(tc.tile_pool(name="sc_oh", bufs=3))
                sap = c3.enter_context(tc.tile_pool(name="sc_sa", bufs=2,
                                                    space="PSUM"))
                thp = c3.enter_context(tc.tile_pool(name="sc_th", bufs=3))
                arp = c3.enter_context(tc.tile_pool(name="sc_ar", bufs=4))
                rwp = c3.enter_context(tc.tile_pool(name="sc_rw", bufs=2,
                                                    space="PSUM"))

                for t in range(N_SIB_TILE):
                    # ---------- sibling tile ----------
                    bucket = t // TILES_PER_BUCKET
                    ccs = (bucket >> 4, (bucket >> 2) & 3, bucket & 3)
                    bc = ip.tile([128, 384], BF16, tag="bc")
                    nc.sync.dma_start(bc[:],
                                      sibidx[t:t + 1, :].broadcast_to((128, 384)))
                    oh = ohp.tile([128, 384], BF16, tag="oh")
                    nc.vector.tensor_scalar(
                        oh[:], bc[:], iota_sb[:, 0:1], None,
                        op0=mybir.AluOpType.is_equal,
                    )
                    acc = sap.tile([128, 512], F32, tag="acc")
                    for r in range(3):
                        nc.tensor.matmul(
                            acc[:],
                            lhsT=oh[:, r * 128:(r + 1) * 128],
                            rhs=tables_sb[:, ccs[r], r * 512:(r + 1) * 512],
                            start=(r == 0), stop=(r == 2),
                        )
                    th = thp.tile([128, 512], BF16, tag="th")
                    nc.scalar.activation(th[:], acc[:], TANH)
                    scr = thp.tile([128, 512], BF16, tag="scr")
                    nc.vector.tensor_mul(scr[:], th[:], wrep[:])
                    nc.vector.tensor_reduce(
                        sib_sb[:, t:t + 1], scr[:],
                        mybir.AxisListType.X, mybir.AluOpType.add,
                    )
                    # ---------- dense arc row ----------
                    if t % 3 == 0:
                        h = t // 3
                        marg = arp.tile([128, 4, 512], BF16, tag="marg")
                        nc.gpsimd.tensor_add(
                            marg.rearrange("p a b -> p (a b)"),
                            modsT.rearrange("p a b -> p (a b)"),
                            dtexp_sb[:, :, 511 - h:1023 - h],
                        )
                        tha = arp.tile([128, 4, 512], BF16, tag="tha")
                        for hc in range(4):
                            nc.scalar.activation(
                                tha[:, hc, :], marg[:, hc, :], TANH,
                                bias=headsb[:, hc, h:h + 1],
                            )
                        row_ps = rwp.tile([1, 512], F32, tag="row")
                        for hc in range(4):
                            nc.tensor.matmul(
                                row_ps[:],
                                lhsT=wT_sb[:, hc:hc + 1],
                                rhs=tha[:, hc, :],
                                start=(hc == 0), stop=(hc == 3),
                            )
                        row_sb = arp.tile([1, 512], F32, tag="rowsb")
                        if h % 2 == 0:
                            nc.vector.tensor_copy(row_sb[:], row_ps[:])
                        else:
                            nc.scalar.copy(row_sb[:], row_ps[:])
                        nc.sync.dma_start(arc_out[h:h + 1, :], row_sb[:])

                nc.sync.dma_start(sib_out[:], sib_sb[:])
    return nc


_CACHE = {}


def _get_program():
    if "nc" not in _CACHE:
        nc = bass.Bass()
        _build(nc)
        _legalize_waits(nc)
        _CACHE["nc"] = nc
    return _CACHE["nc"]


def _host_prepare(inputs):
    import ml_dtypes

    f32 = np.float32

    def bf(x):
        return np.asarray(x, f32).astype(ml_dtypes.bfloat16)

    words = np.asarray(inputs["words"]).astype(np.int64)
    tags = np.asarray(inputs["tags"]).astype(np.int64)
    word_emb = np.asarray(inputs["word_emb"], f32)
    tag_emb = np.asarray(inputs["tag_emb"], f32)
    emb = np.concatenate([word_emb[words], tag_emb[tags]], axis=-1)  # [512, 364]
    emb_aug = np.concatenate([emb, np.ones((S, 1), f32)], axis=1)    # [512, 365]

    def packT(x, rows):  # -> [rows(pad), S] = x.T zero-padded
        out = np.zeros((rows, x.shape[0]), f32)
        out[: x.shape[1]] = x.T
        return out

    def chunkP(x):  # [K*128, ...] -> [128, K, ...]
        sh = x.shape
        return x.reshape(sh[0] // 128, 128, *sh[1:]).transpose(
            1, 0, *range(2, x.ndim + 1)).copy()

    embT_f = bf(chunkP(packT(emb_aug, 384)))
    embT_b = bf(chunkP(packT(emb_aug[::-1], 384)))

    def wih_pack(Wih, bih, bhh, kdim, rows):
        out = np.zeros((rows, 2, 4 * H), f32)
        for d in range(2):
            out[:kdim, d] = np.asarray(Wih[d], f32).T
            out[kdim, d] = np.asarray(bih[d], f32) + np.asarray(bhh[d], f32)
        return out

    wih0T = bf(chunkP(wih_pack(inputs["Wih0"], inputs["bih0"], inputs["bhh0"],
                               364, 384)))
    wih1T = bf(chunkP(wih_pack(inputs["Wih1"], inputs["bih1"], inputs["bhh1"],
                               1024, 1152)))

    def whh_pack(Whh):
        out = np.zeros((128, 4, 2, 4 * H), f32)
        for d in range(2):
            wt = np.asarray(Whh[d], f32).T  # [512 k, 2048 g]
            out[:, :, d, :] = wt.reshape(4, 128, 4 * H).transpose(1, 0, 2)
        return out

    whh0T = bf(whh_pack(inputs["Whh0"]))
    whh1T = bf(whh_pack(inputs["Whh1"]))

    projs = [inputs["head_W"], inputs["mod_W"], inputs["sib_head_W"],
             inputs["sib_mod_W"], inputs["sib_sib_W"]]
    projT = np.zeros((1152, 5 * H), f32)
    for i, W in enumerate(projs):
        projT[:1024, i * H:(i + 1) * H] = np.asarray(W, f32).T
    projT = bf(chunkP(projT))

    # distance table expanded over offsets delta = m - h in [-511, 511]
    D = (np.asarray(inputs["dist_emb"], f32) @ np.asarray(inputs["dist_W"], f32).T
         + np.asarray(inputs["dist_b"], f32))          # [34, H]
    delta = np.arange(-511, 512)
    bidx = np.searchsorted(BINS, np.abs(delta), side="right") - 1
    didx = np.where(delta > 0, bidx, bidx + NB)
    DTg = D[didx]                                       # [1023, H]

    w = np.asarray(inputs["arc_w"], f32).reshape(512)
    wT = bf(w.reshape(4, 128).T)                        # [128, 4]
    wrep = bf(np.tile(w[None, :], (128, 1)))            # [128, 512]

    iota = np.arange(128, dtype=f32).reshape(128, 1)
    mask = np.zeros((128, 2), f32)
    for mi, s in enumerate((7, 15)):
        c = np.arange(64)
        v = ((8 * c + s) > (K_WARM - 1)).astype(f32)
        mask[0:64, mi] = v
        mask[64:128, mi] = v
    revm = np.zeros((128, 128), f32)
    revm[np.arange(128), 127 - np.arange(128)] = 1.0

    base = {
        "embT_f": embT_f, "embT_b": embT_b,
        "wih0T": wih0T, "whh0T": whh0T, "wih1T": wih1T, "whh1T": whh1T,
        "projT": projT, "wT_in": wT, "wrep_in": wrep,
        "iota_in": iota, "mask_in": mask, "rev_in": bf(revm),
    }

    sh_i = np.asarray(inputs["sib_head"]).astype(np.int64)
    sm_i = np.asarray(inputs["sib_mod"]).astype(np.int64)
    ss_i = np.asarray(inputs["sib_sib"]).astype(np.int64)
    per_core = ASIB // NC

    in_maps = []
    slotmaps = []
    for core in range(NC):
        m = dict(base)
        # shifted distance window: dtexp_core[j] = DTg[j - 64*core]
        dte = np.zeros((1023, 512), f32)
        lo = 64 * core
        dte[lo:] = DTg[: 1023 - lo]
        m["dtexp"] = bf(chunkP(dte.T.copy()))           # [128, 4, 1023]
        # heads-column selector: one-hot of position 64*core + j
        sel = np.zeros((512, 64), f32)
        sel[np.arange(64) + 64 * core, np.arange(64)] = 1.0
        m["selT"] = bf(chunkP(sel))                     # [128, 4, 64]
        # bucket-sorted sibling tiles
        s0 = core * per_core
        hh = sh_i[s0:s0 + per_core]
        mm_ = sm_i[s0:s0 + per_core]
        ss_ = ss_i[s0:s0 + per_core]
        bucket = (hh // 128) * 16 + (mm_ // 128) * 4 + (ss_ // 128)
        order = np.argsort(bucket, kind="stable")
        counts = np.bincount(bucket, minlength=N_BUCKET)
        if counts.max() > TILES_PER_BUCKET * 128:
            raise RuntimeError(f"sib bucket overflow: {counts.max()}")
        idx_rows = np.zeros((N_SIB_TILE, 3, 128), f32)
        slotmap = np.full(N_SIB_TILE * 128, -1, np.int64)
        pos = 0
        for b in range(N_BUCKET):
            n = counts[b]
            sel_idx = order[pos:pos + n]
            pos += n
            t0 = b * TILES_PER_BUCKET
            slot = np.arange(n)
            tt = t0 + slot // 128
            pp = slot % 128
            idx_rows[tt, 0, pp] = hh[sel_idx] - 128 * (b >> 4)
            idx_rows[tt, 1, pp] = mm_[sel_idx] - 128 * ((b >> 2) & 3)
            idx_rows[tt, 2, pp] = ss_[sel_idx] - 128 * (b & 3)
            slotmap[tt * 128 + pp] = s0 + sel_idx
        m["sibidx"] = bf(idx_rows.reshape(N_SIB_TILE, 384))
        in_maps.append(m)
        slotmaps.append(slotmap)
    return in_maps, slotmaps


LAST_EXEC_NS = None


def kernel(**inputs):
    global LAST_EXEC_NS
    _install_ntff_hook()
    from concourse.bass_utils import run_bass_kernel_spmd

    nc = _get_program()
    in_maps, slotmaps = _host_prepare(inputs)
    import os

    trace = os.environ.get("KERNEL_TRACE", "0") == "1"
    res = run_bass_kernel_spmd(nc, in_maps, list(range(NC)), trace=trace)
    LAST_EXEC_NS = res.exec_time_ns
    _CACHE["res"] = res

    ah = np.asarray(inputs["arc_head"]).astype(np.int64)
    am = np.asarray(inputs["arc_mod"]).astype(np.int64)
    Smat = np.zeros((S, S), np.float32)
    sib_scores = np.zeros(ASIB, np.float32)
    for core in range(NC):
        r = res.results[core]
        Smat[core * N_ARC_H:(core + 1) * N_ARC_H] = np.asarray(r["arc_out"])
        sib_flat = np.asarray(r["sib_out"]).T.reshape(-1)  # [tile*128]
        sm = slotmaps[core]
        valid = sm >= 0
        sib_scores[sm[valid]] = sib_flat[valid]
    arc_scores = Smat[ah, am]
    return np.concatenate([arc_scores, sib_scores])


# revision 25
# speedup vs baseline: 1.1655x; 1.1655x over previous
"""Trainium2 Bass kernel for nn_DependencyNeuralModel (dependency parser scorer).

Device strategy (8 NeuronCores, SPMD):
  - Encoder (2-layer BiLSTM over S=512) replicated on every core,
    chunk-parallel: 64 chunks x 2 dirs = 128 batch rows advance in
    lock-step; each chunk warms up from zero over K=16 positions
    (forget-gate contraction makes the warmup error ~6e-3).
    Per step the wx term is injected into PSUM by an identity matmul and
    the gate nonlinearities read PSUM directly, one 512-wide gate region
    at a time, so ACT overlaps the PE stream.
  - Arc scores: A = S^2 exactly, so the full score matrix
    Score[h,m] = w . tanh(headsT[:,h] + modsT[:,m] + DT[:, m-h]) is
    computed densely with NO gather: partition dim = hidden (4 chunks),
    mods as a resident tile, heads column as the ACT bias, and the
    distance embedding as a sliding window into an offset table.
    h rows are sharded across cores (64 per core); the host does the
    final Score[arc_head, arc_mod] scalar fancy-index.
  - Sibling scores: parts bucket-sorted on host by
    (head//128, mod//128, sib//128) into 64 buckets x 3 static tiles;
    each tile needs just 3 one-hot gather matmuls (one 128-row chunk per
    role).  tanh on ACT, fused multiply+reduce on DVE.
Host does index/layout preparation, the final arc fancy-index and
sibling unpermute.
"""
import sys
import types

import numpy as np

sys.path.insert(0, "/opt/trn_rl_repo")

import concourse.bass as bass
import concourse.mybir as mybir
from concourse.tile import TileContext
from concourse.masks import make_identity

S = 512
H = 512
A = 262144
ASIB = 131072
NB = 17
L = 8
K_WARM = 16
NSTEP = K_WARM + L  # 24
NC = 8
F32 = mybir.dt.float32
BF16 = mybir.dt.bfloat16
BINS = np.array(list(range(10)) + list(range(10, 40, 5)) + [40], dtype=np.int64)

N_BUCKET = 64          # (head//128, mod//128, sib//128)
TILES_PER_BUCKET = 3   # 384 slots per bucket; max observed occupancy ~306
N_SIB_TILE = N_BUCKET * TILES_PER_BUCKET  # 192
N_ARC_H = S // NC      # 64 dense score-matrix rows per core
MASK_STEPS = {7: 0, 15: 1}


def _install_ntff_hook():
    if "antenv.axon_hooks" in sys.modules:
        return
    mod = types.ModuleType("antenv.axon_hooks")
    state = {"hook": None, "tried": False}

    def set_axon_ntff_profile_hook(hook):
        state["hook"] = hook

    def get_axon_ntff_profile_hook():
        if state["hook"] is None and not state["tried"]:
            state["tried"] = True
            try:
                from trn_agent_boot.trn_boot import _ntff_profile_via_ctypes

                state["hook"] = _ntff_profile_via_ctypes("/opt/axon/libaxon_pjrt.so")
            except Exception:
                state["hook"] = None
        return state["hook"]

    mod.set_axon_ntff_profile_hook = set_axon_ntff_profile_hook
    mod.get_axon_ntff_profile_hook = get_axon_ntff_profile_hook
    import antenv

    antenv.axon_hooks = mod
    sys.modules["antenv.axon_hooks"] = mod


def _legalize_waits(nc):
    """This walrus accepts at most one semaphore wait per instruction;
    split extra waits onto same-engine NOPs placed just before."""
    ctr = [0]
    for f in nc.m.functions:
        for blk in f.blocks:
            out = []
            dirty = False
            for ins in blk.instructions:
                si = ins.sync_info
                if si is not None and si.on_wait and len(si.on_wait) > 1:
                    waits = list(si.on_wait)
                    for w in waits[:-1]:
                        ctr[0] += 1
                        nop = mybir.InstNoOp(name=f"waitfix-{ctr[0]}")
                        nop.engine = ins.engine
                        nop.sync_info = mybir.SyncInfo(on_wait=[w], on_update=[])
                        out.append(nop)
                    ins.sync_info = mybir.SyncInfo(
                        on_wait=[waits[-1]],
                        on_update=list(si.on_update) if si.on_update else [],
                    )
                    dirty = True
                out.append(ins)
            if dirty:
                blk.instructions = out
    return nc


def _lstm_layer(nc, tc, ident, mask_sb, whh_sb, wx_dram, f_dram, b_dram):
    """One BiLSTM layer, chunk-parallel.  128 batch rows: partitions 0:64
    dir0 chunks, 64:128 dir1.  Gate regions (512 cols each) get their own
    PSUM tile; wx is injected by an identity matmul so ACT reads PSUM."""
    import contextlib

    SIG = mybir.ActivationFunctionType.Sigmoid
    TANH = mybir.ActivationFunctionType.Tanh

    with contextlib.ExitStack() as ctx:
        wxp = ctx.enter_context(tc.tile_pool(name="lstm_wx", bufs=2))
        pg = ctx.enter_context(tc.tile_pool(name="lstm_pg", bufs=6, space="PSUM"))
        gp = ctx.enter_context(tc.tile_pool(name="lstm_g", bufs=6))
        cp = ctx.enter_context(tc.tile_pool(name="lstm_c", bufs=8))
        pst = ctx.enter_context(tc.tile_pool(name="lstm_pst", bufs=2, space="PSUM"))
        st = ctx.enter_context(tc.tile_pool(name="lstm_state", bufs=1))

        h_t = st.tile([128, 4, 128], BF16)  # h transposed: [k-part, kc, b]
        c_st = st.tile([128, 512], F32)
        nc.vector.memset(h_t.rearrange("p a b -> p (a b)"), 0.0)
        nc.vector.memset(c_st[:], 0.0)

        for s in range(NSTEP):
            wx = wxp.tile([128, 2048], BF16, tag="wx")
            for d in range(2):
                nc.sync.dma_start(
                    wx[d * 64:(d + 1) * 64, :], wx_dram[d, s:s + 505:8, :]
                )
            # all four wx-injection matmuls first: they depend only on wx, so
            # they fill the PE gap while the previous step's tail completes
            P = {}
            for ng in (1, 0, 2, 3):  # f, i, g, o
                Pt = pg.tile([128, 512], F32, tag="P")
                P[ng] = Pt
                nc.tensor.matmul(
                    Pt[:], lhsT=ident[:], rhs=wx[:, ng * 512:(ng + 1) * 512],
                    start=True, stop=False, skip_group_check=True,
                )
            gate = {}
            for ng in (1, 0, 2, 3):
                for d in range(2):
                    bs = slice(d * 64, (d + 1) * 64)
                    for kc in range(4):
                        nc.tensor.matmul(
                            P[ng][bs, :],
                            lhsT=h_t[:, kc, bs],
                            rhs=whh_sb[:, kc, d, ng * 512:(ng + 1) * 512],
                            start=False,
                            stop=(d == 1 and kc == 3),
                            skip_group_check=True,
                        )
                g = gp.tile([128, 512], BF16, tag=f"g{ng}")
                nc.scalar.activation(g[:], P[ng][:], TANH if ng == 2 else SIG)
                gate[ng] = g
            t1 = cp.tile([128, 512], F32, tag="t1")
            nc.vector.tensor_mul(t1[:], gate[1][:], c_st[:])
            t2 = cp.tile([128, 512], F32, tag="t2")
            nc.vector.tensor_mul(t2[:], gate[0][:], gate[2][:])
            nc.vector.tensor_add(c_st[:], t1[:], t2[:])
            # tail in halves so transposes/copies overlap the second tanh
            h_new = cp.tile([128, 512], BF16, tag="h")
            mi = MASK_STEPS.get(s)
            for half in range(2):
                sl = slice(half * 256, (half + 1) * 256)
                tch = cp.tile([128, 256], BF16, tag=f"tch{half}")
                nc.scalar.activation(tch[:], c_st[:, sl], TANH)
                nc.vector.tensor_mul(h_new[:, sl], gate[3][:, sl], tch[:])
                if mi is not None:
                    nc.vector.tensor_scalar_mul(h_new[:, sl], h_new[:, sl],
                                                mask_sb[:, mi:mi + 1])
                for kc in (2 * half, 2 * half + 1):
                    tp = pst.tile([128, 128], BF16, tag="tr")
                    nc.tensor.transpose(tp[:], h_new[:, kc * 128:(kc + 1) * 128],
                                        ident[:])
                    nc.vector.tensor_copy(h_t[:, kc, :], tp[:])
            if mi is not None:
                nc.vector.tensor_scalar_mul(c_st[:], c_st[:], mask_sb[:, mi:mi + 1])
            if s >= K_WARM:
                o = s - K_WARM
                nc.sync.dma_start(f_dram[o:505 + o:8, :], h_new[0:64, :])
                nc.sync.dma_start(b_dram[o:505 + o:8, :], h_new[64:128, :])


def _transpose_pair(nc, tc, ident, rev, f_dram, b_dram, dstT, dstTrev, one_row):
    """Build [feat, pos] lhsT chunks (and optionally pos-reversed copy) from
    the per-direction output buffers.  dstT/dstTrev: [128, 9, 512] tiles;
    chunk 8 row 0 is set to ones (bias); rest of chunk 8 zero."""
    import contextlib

    with contextlib.ExitStack() as ctx:
        sb = ctx.enter_context(tc.tile_pool(name="tp_sb", bufs=3))
        ps = ctx.enter_context(tc.tile_pool(name="tp_ps", bufs=2, space="PSUM"))
        for dst in (dstT, dstTrev):
            if dst is None:
                continue
            nc.vector.memset(dst[:, 8, :], 0.0)
            nc.vector.tensor_copy(dst[0:1, 8, :], one_row[:])
        for pc in range(4):
            fsrc = sb.tile([128, 512], BF16, tag="fsrc")
            nc.sync.dma_start(fsrc[:], f_dram[pc * 128:(pc + 1) * 128, :])
            bsrc = sb.tile([128, 512], BF16, tag="bsrc")
            nc.sync.dma_start(bsrc[:], b_dram[pc * 128:(pc + 1) * 128, :])
            for j in range(4):
                fs = fsrc[:, j * 128:(j + 1) * 128]
                bs = bsrc[:, j * 128:(j + 1) * 128]
                tp = ps.tile([128, 128], BF16, tag="tp")
                nc.tensor.transpose(tp[:], fs, ident[:])
                nc.vector.tensor_copy(dstT[:, j, pc * 128:(pc + 1) * 128], tp[:])
                if dstTrev is not None:
                    tpr = ps.tile([128, 128], BF16, tag="tpr")
                    nc.tensor.transpose(tpr[:], fs, rev[:])
                    nc.vector.tensor_copy(
                        dstTrev[:, j, (3 - pc) * 128:(4 - pc) * 128], tpr[:])
                # b rows are scan order q; position = 511-q: reverse via rev
                tpb = ps.tile([128, 128], BF16, tag="tpb")
                nc.tensor.transpose(tpb[:], bs, rev[:])
                nc.vector.tensor_copy(
                    dstT[:, 4 + j, (3 - pc) * 128:(4 - pc) * 128], tpb[:])
                if dstTrev is not None:
                    tpb2 = ps.tile([128, 128], BF16, tag="tpb2")
                    nc.tensor.transpose(tpb2[:], bs, ident[:])
                    nc.vector.tensor_copy(
                        dstTrev[:, 4 + j, pc * 128:(pc + 1) * 128], tpb2[:])


def _input_gemm(nc, tc, lhsT_tiles, w_sb, wx_dram, nk):
    """WX[d] = lhsT_d.T @ w[d] -> wx_dram[d, 16:528, :], bf16.
    lhsT_tiles: per-dir [128, nk, 512] SBUF ([feat-part, chunk, pos]).
    w_sb: [128, nk, 2, 2048] SBUF weights."""
    import contextlib

    with contextlib.ExitStack() as ctx:
        sb = ctx.enter_context(tc.tile_pool(name="ig_sb", bufs=4))
        ps = ctx.enter_context(tc.tile_pool(name="ig_ps", bufs=4, space="PSUM"))
        for d in range(2):
            lhsT = lhsT_tiles[d]
            for mc in range(4):
                for ngc in range(4):
                    acc = ps.tile([128, 512], F32, tag="acc")
                    for kc in range(nk):
                        nc.tensor.matmul(
                            acc[:],
                            lhsT=lhsT[:, kc, mc * 128:(mc + 1) * 128],
                            rhs=w_sb[:, kc, d, ngc * 512:(ngc + 1) * 512],
                            start=(kc == 0),
                            stop=(kc == nk - 1),
                        )
                    osb = sb.tile([128, 512], BF16, tag="osb")
                    if (mc + ngc) % 2 == 0:
                        nc.vector.tensor_copy(osb[:], acc[:])
                    else:
                        nc.scalar.copy(osb[:], acc[:])
                    nc.sync.dma_start(
                        wx_dram[d, 16 + mc * 128:16 + (mc + 1) * 128,
                                ngc * 512:(ngc + 1) * 512],
                        osb[:],
                    )


def _build(nc):
    TANH = mybir.ActivationFunctionType.Tanh
    embT_f = nc.dram_tensor("embT_f", [128, 3, 512], BF16, kind="ExternalInput")
    embT_b = nc.dram_tensor("embT_b", [128, 3, 512], BF16, kind="ExternalInput")
    wih0T = nc.dram_tensor("wih0T", [128, 3, 2, 2048], BF16, kind="ExternalInput")
    whh0T = nc.dram_tensor("whh0T", [128, 4, 2, 2048], BF16, kind="ExternalInput")
    wih1T = nc.dram_tensor("wih1T", [128, 9, 2, 2048], BF16, kind="ExternalInput")
    whh1T = nc.dram_tensor("whh1T", [128, 4, 2, 2048], BF16, kind="ExternalInput")
    projT = nc.dram_tensor("projT", [128, 9, 2560], BF16, kind="ExternalInput")
    dtexp = nc.dram_tensor("dtexp", [128, 4, 1023], BF16, kind="ExternalInput")
    selT = nc.dram_tensor("selT", [128, 4, 64], BF16, kind="ExternalInput")
    wT_in = nc.dram_tensor("wT_in", [128, 4], BF16, kind="ExternalInput")
    wrep_in = nc.dram_tensor("wrep_in", [128, 512], BF16, kind="ExternalInput")
    sib_oh = nc.dram_tensor("sib_oh", [128, N_SIB_TILE * 384], BF16,
                            kind="ExternalInput")
    mask_in = nc.dram_tensor("mask_in", [128, 2], F32, kind="ExternalInput")
    rev_in = nc.dram_tensor("rev_in", [128, 128], BF16, kind="ExternalInput")
    arc_out = nc.dram_tensor("arc_out", [N_ARC_H, 512], F32, kind="ExternalOutput")
    sib_out = nc.dram_tensor("sib_out", [128, N_SIB_TILE], F32,
                             kind="ExternalOutput")

    wx0 = nc.dram_tensor("wx0", [2, 528, 2048], BF16)
    wx1 = nc.dram_tensor("wx1", [2, 528, 2048], BF16)
    f0d = nc.dram_tensor("f0d", [512, 512], BF16)
    b0d = nc.dram_tensor("b0d", [512, 512], BF16)
    f1d = nc.dram_tensor("f1d", [512, 512], BF16)
    b1d = nc.dram_tensor("b1d", [512, 512], BF16)

    import contextlib

    with TileContext(nc) as tc:
        with contextlib.ExitStack() as ctx:
            const = ctx.enter_context(tc.tile_pool(name="const", bufs=1))
            enc = ctx.enter_context(tc.tile_pool(name="enc", bufs=1))
            sco = ctx.enter_context(tc.tile_pool(name="sco", bufs=1))

            ident = const.tile([128, 128], BF16)
            make_identity(nc, ident[:])
            rev = const.tile([128, 128], BF16)
            nc.sync.dma_start(rev[:], rev_in[:])
            mask_sb = const.tile([128, 2], F32)
            nc.sync.dma_start(mask_sb[:], mask_in[:])
            wT_sb = const.tile([128, 4], BF16)
            nc.sync.dma_start(wT_sb[:], wT_in[:])
            wrep = const.tile([128, 512], BF16)
            nc.sync.dma_start(wrep[:], wrep_in[:])
            sel_sb = const.tile([128, 4, 64], BF16)
            nc.sync.dma_start(sel_sb.rearrange("p a b -> p (a b)"),
                              selT.rearrange("p a b -> p (a b)"))
            one_row = const.tile([1, 512], BF16)
            nc.vector.memset(one_row[:], 1.0)

            # zero-pad warmup rows of WX buffers
            with tc.tile_pool(name="zp", bufs=1) as zp:
                zrow = zp.tile([16, 2048], BF16)
                nc.vector.memset(zrow[:], 0.0)
                for wxd in (wx0, wx1):
                    for d in range(2):
                        nc.sync.dma_start(wxd[d, 0:16, :], zrow[:])

            # ---- WX0 ----
            with tc.tile_pool(name="w0", bufs=1) as w0p:
                wih0_sb = w0p.tile([128, 3, 2, 2048], BF16)
                nc.sync.dma_start(wih0_sb.rearrange("p a b c -> p (a b c)"),
                                  wih0T.rearrange("p a b c -> p (a b c)"))
                ef = w0p.tile([128, 3, 512], BF16)
                nc.sync.dma_start(ef.rearrange("p a b -> p (a b)"),
                                  embT_f.rearrange("p a b -> p (a b)"))
                eb = w0p.tile([128, 3, 512], BF16)
                nc.sync.dma_start(eb.rearrange("p a b -> p (a b)"),
                                  embT_b.rearrange("p a b -> p (a b)"))
                _input_gemm(nc, tc, [ef, eb], wih0_sb, wx0, 3)

                # ---- layer 0 (whh0 shares this scope's lifetime) ----
                whh0_sb = w0p.tile([128, 4, 2, 2048], BF16)
                nc.sync.dma_start(whh0_sb.rearrange("p a b c -> p (a b c)"),
                                  whh0T.rearrange("p a b c -> p (a b c)"))
                _lstm_layer(nc, tc, ident, mask_sb, whh0_sb, wx0, f0d, b0d)

            # ---- x1T / x1Trev ----
            x1T = enc.tile([128, 9, 512], BF16, tag="x1T")
            x1Trev = enc.tile([128, 9, 512], BF16, tag="x1Trev")
            _transpose_pair(nc, tc, ident, rev, f0d, b0d, x1T, x1Trev, one_row)

            # ---- WX1 + layer 1 ----
            with tc.tile_pool(name="w1", bufs=1) as w1p:
                wih1_sb = w1p.tile([128, 9, 2, 2048], BF16)
                nc.sync.dma_start(wih1_sb.rearrange("p a b c -> p (a b c)"),
                                  wih1T.rearrange("p a b c -> p (a b c)"))
                _input_gemm(nc, tc, [x1T, x1Trev], wih1_sb, wx1, 9)
            with tc.tile_pool(name="w1b", bufs=1) as w1bp:
                whh1_sb = w1bp.tile([128, 4, 2, 2048], BF16)
                nc.sync.dma_start(whh1_sb.rearrange("p a b c -> p (a b c)"),
                                  whh1T.rearrange("p a b c -> p (a b c)"))
                _lstm_layer(nc, tc, ident, mask_sb, whh1_sb, wx1, f1d, b1d)

            # ---- statesT ----
            stT = enc.tile([128, 9, 512], BF16, tag="x1T")  # reuse slot
            _transpose_pair(nc, tc, ident, rev, f1d, b1d, stT, None, one_row)

            # ---- projection tables ----
            tables_sb = sco.tile([128, 4, 1536], BF16, tag="tables")
            heads_pos = sco.tile([128, 4, 512], BF16, tag="heads_pos")
            modsT = sco.tile([128, 4, 512], BF16, tag="modsT")
            headsb = sco.tile([128, 4, 64], F32, tag="headsb")
            with contextlib.ExitStack() as c2:
                pj = c2.enter_context(tc.tile_pool(name="pj", bufs=1))
                ps2 = c2.enter_context(tc.tile_pool(name="tb_ps", bufs=5,
                                                    space="PSUM"))
                ps2b = c2.enter_context(tc.tile_pool(name="tb_ps2", bufs=1,
                                                     space="PSUM"))
                ps2c = c2.enter_context(tc.tile_pool(name="tb_ps3", bufs=2,
                                                     space="PSUM"))
                projT_sb = pj.tile([128, 9, 2560], BF16)
                nc.sync.dma_start(projT_sb.rearrange("p a b -> p (a b)"),
                                  projT.rearrange("p a b -> p (a b)"))
                # sib tables + heads in pos-part layout
                for mc in range(4):
                    for r in range(4):  # 0..2 sib tables, 3 = heads
                        toff = (2 + r) * 512 if r < 3 else 0
                        acc = ps2.tile([128, 512], F32, tag="acc")
                        for kc in range(9):
                            nc.tensor.matmul(
                                acc[:],
                                lhsT=stT[:, kc, mc * 128:(mc + 1) * 128],
                                rhs=projT_sb[:, kc, toff:toff + 512],
                                start=(kc == 0), stop=(kc == 8),
                            )
                        if r < 3:
                            nc.vector.tensor_copy(
                                tables_sb[:, mc, r * 512:(r + 1) * 512], acc[:])
                        else:
                            nc.scalar.copy(heads_pos[:, mc, :], acc[:])
                # mods in hidden-part layout
                for hc in range(4):
                    acc = ps2.tile([128, 512], F32, tag="acc")
                    for kc in range(9):
                        nc.tensor.matmul(
                            acc[:],
                            lhsT=projT_sb[:, kc, 512 + hc * 128:512 + hc * 128 + 128],
                            rhs=stT[:, kc, :],
                            start=(kc == 0), stop=(kc == 8),
                        )
                    nc.vector.tensor_copy(modsT[:, hc, :], acc[:])
                # per-core heads columns: select 64 pos, then transpose
                hsel_ps = ps2b.tile([64, 512], F32, tag="hsel")
                for mc in range(4):
                    nc.tensor.matmul(
                        hsel_ps[:], lhsT=sel_sb[:, mc, :], rhs=heads_pos[:, mc, :],
                        start=(mc == 0), stop=(mc == 3),
                    )
                hsel_sb = pj.tile([64, 512], BF16)
                nc.scalar.copy(hsel_sb[:], hsel_ps[:])
                for hc in range(4):
                    tp = ps2c.tile([128, 64], BF16, tag="htp")
                    nc.tensor.transpose(tp[:], hsel_sb[:, hc * 128:(hc + 1) * 128],
                                        ident[0:64, 0:64])
                    nc.vector.tensor_copy(headsb[:, hc, :], tp[:])

            dtexp_sb = sco.tile([128, 4, 1023], BF16, tag="dtexp")
            nc.sync.dma_start(dtexp_sb.rearrange("p a b -> p (a b)"),
                              dtexp.rearrange("p a b -> p (a b)"))
            sib_sb = sco.tile([128, N_SIB_TILE], F32, tag="sib_sb")

            # ---- scoring: interleave sib tiles (PE/DVE) with arc rows (ACT) --
            with contextlib.ExitStack() as c3:
                ohp = c3.enter_context(tc.tile_pool(name="sc_oh", bufs=4))
                sap = c3.enter_context(tc.tile_pool(name="sc_sa", bufs=2,
                                                    space="PSUM"))
                thp = c3.enter_context(tc.tile_pool(name="sc_th", bufs=3))
                arp = c3.enter_context(tc.tile_pool(name="sc_ar", bufs=4))
                rwp = c3.enter_context(tc.tile_pool(name="sc_rw", bufs=2,
                                                    space="PSUM"))

                for t in range(N_SIB_TILE):
                    # ---------- sibling tile ----------
                    bucket = t // TILES_PER_BUCKET
                    ccs = (bucket >> 4, (bucket >> 2) & 3, bucket & 3)
                    oh = ohp.tile([128, 384], BF16, tag="oh")
                    nc.sync.dma_start(oh[:], sib_oh[:, t * 384:(t + 1) * 384])
                    acc = sap.tile([128, 512], F32, tag="acc")
                    for r in range(3):
                        nc.tensor.matmul(
                            acc[:],
                            lhsT=oh[:, r * 128:(r + 1) * 128],
                            rhs=tables_sb[:, ccs[r], r * 512:(r + 1) * 512],
                            start=(r == 0), stop=(r == 2),
                        )
                    th = thp.tile([128, 512], BF16, tag="th")
                    nc.scalar.activation(th[:], acc[:], TANH)
                    scr = thp.tile([128, 512], BF16, tag="scr")
                    nc.vector.tensor_mul(scr[:], th[:], wrep[:])
                    nc.vector.tensor_reduce(
                        sib_sb[:, t:t + 1], scr[:],
                        mybir.AxisListType.X, mybir.AluOpType.add,
                    )
                    # ---------- dense arc row ----------
                    if t % 3 == 0:
                        h = t // 3
                        marg = arp.tile([128, 4, 512], BF16, tag="marg")
                        nc.gpsimd.tensor_add(
                            marg.rearrange("p a b -> p (a b)"),
                            modsT.rearrange("p a b -> p (a b)"),
                            dtexp_sb[:, :, 511 - h:1023 - h],
                        )
                        tha = arp.tile([128, 4, 512], BF16, tag="tha")
                        for hc in range(4):
                            nc.scalar.activation(
                                tha[:, hc, :], marg[:, hc, :], TANH,
                                bias=headsb[:, hc, h:h + 1],
                            )
                        row_ps = rwp.tile([1, 512], F32, tag="row")
                        for hc in range(4):
                            nc.tensor.matmul(
                                row_ps[:],
                                lhsT=wT_sb[:, hc:hc + 1],
                                rhs=tha[:, hc, :],
                                start=(hc == 0), stop=(hc == 3),
                            )
                        row_sb = arp.tile([1, 512], F32, tag="rowsb")
                        if h % 2 == 0:
                            nc.vector.tensor_copy(row_sb[:], row_ps[:])
                        else:
                            nc.scalar.copy(row_sb[:], row_ps[:])
                        nc.sync.dma_start(arc_out[h:h + 1, :], row_sb[:])

                nc.sync.dma_start(sib_out[:], sib_sb[:])
    return nc


_CACHE = {}


def _get_program():
    if "nc" not in _CACHE:
        nc = bass.Bass()
        _build(nc)
        _legalize_waits(nc)
        _CACHE["nc"] = nc
    return _CACHE["nc"]


def _host_prepare(inputs):
    import ml_dtypes

    f32 = np.float32

    def bf(x):
        return np.asarray(x, f32).astype(ml_dtypes.bfloat16)

    words = np.asarray(inputs["words"]).astype(np.int64)
    tags = np.asarray(inputs["tags"]).astype(np.int64)
    word_emb = np.asarray(inputs["word_emb"], f32)
    tag_emb = np.asarray(inputs["tag_emb"], f32)
    emb = np.concatenate([word_emb[words], tag_emb[tags]], axis=-1)  # [512, 364]
    emb_aug = np.concatenate([emb, np.ones((S, 1), f32)], axis=1)    # [512, 365]

    def packT(x, rows):  # -> [rows(pad), S] = x.T zero-padded
        out = np.zeros((rows, x.shape[0]), f32)
        out[: x.shape[1]] = x.T
        return out

    def chunkP(x):  # [K*128, ...] -> [128, K, ...]
        sh = x.shape
        return x.reshape(sh[0] // 128, 128, *sh[1:]).transpose(
            1, 0, *range(2, x.ndim + 1)).copy()

    embT_f = bf(chunkP(packT(emb_aug, 384)))
    embT_b = bf(chunkP(packT(emb_aug[::-1], 384)))

    def wih_pack(Wih, bih, bhh, kdim, rows):
        out = np.zeros((rows, 2, 4 * H), f32)
        for d in range(2):
            out[:kdim, d] = np.asarray(Wih[d], f32).T
            out[kdim, d] = np.asarray(bih[d], f32) + np.asarray(bhh[d], f32)
        return out

    wih0T = bf(chunkP(wih_pack(inputs["Wih0"], inputs["bih0"], inputs["bhh0"],
                               364, 384)))
    wih1T = bf(chunkP(wih_pack(inputs["Wih1"], inputs["bih1"], inputs["bhh1"],
                               1024, 1152)))

    def whh_pack(Whh):
        out = np.zeros((128, 4, 2, 4 * H), f32)
        for d in range(2):
            wt = np.asarray(Whh[d], f32).T  # [512 k, 2048 g]
            out[:, :, d, :] = wt.reshape(4, 128, 4 * H).transpose(1, 0, 2)
        return out

    whh0T = bf(whh_pack(inputs["Whh0"]))
    whh1T = bf(whh_pack(inputs["Whh1"]))

    projs = [inputs["head_W"], inputs["mod_W"], inputs["sib_head_W"],
             inputs["sib_mod_W"], inputs["sib_sib_W"]]
    projT = np.zeros((1152, 5 * H), f32)
    for i, W in enumerate(projs):
        projT[:1024, i * H:(i + 1) * H] = np.asarray(W, f32).T
    projT = bf(chunkP(projT))

    # distance table expanded over offsets delta = m - h in [-511, 511]
    D = (np.asarray(inputs["dist_emb"], f32) @ np.asarray(inputs["dist_W"], f32).T
         + np.asarray(inputs["dist_b"], f32))          # [34, H]
    delta = np.arange(-511, 512)
    bidx = np.searchsorted(BINS, np.abs(delta), side="right") - 1
    didx = np.where(delta > 0, bidx, bidx + NB)
    DTg = D[didx]                                       # [1023, H]

    w = np.asarray(inputs["arc_w"], f32).reshape(512)
    wT = bf(w.reshape(4, 128).T)                        # [128, 4]
    wrep = bf(np.tile(w[None, :], (128, 1)))            # [128, 512]

    mask = np.zeros((128, 2), f32)
    for mi, s in enumerate((7, 15)):
        c = np.arange(64)
        v = ((8 * c + s) > (K_WARM - 1)).astype(f32)
        mask[0:64, mi] = v
        mask[64:128, mi] = v
    revm = np.zeros((128, 128), f32)
    revm[np.arange(128), 127 - np.arange(128)] = 1.0

    base = {
        "embT_f": embT_f, "embT_b": embT_b,
        "wih0T": wih0T, "whh0T": whh0T, "wih1T": wih1T, "whh1T": whh1T,
        "projT": projT, "wT_in": wT, "wrep_in": wrep,
        "mask_in": mask, "rev_in": bf(revm),
    }

    sh_i = np.asarray(inputs["sib_head"]).astype(np.int64)
    sm_i = np.asarray(inputs["sib_mod"]).astype(np.int64)
    ss_i = np.asarray(inputs["sib_sib"]).astype(np.int64)
    per_core = ASIB // NC

    in_maps = []
    slotmaps = []
    for core in range(NC):
        m = dict(base)
        # shifted distance window: dtexp_core[j] = DTg[j - 64*core]
        dte = np.zeros((1023, 512), f32)
        lo = 64 * core
        dte[lo:] = DTg[: 1023 - lo]
        m["dtexp"] = bf(chunkP(dte.T.copy()))           # [128, 4, 1023]
        # heads-column selector: one-hot of position 64*core + j
        sel = np.zeros((512, 64), f32)
        sel[np.arange(64) + 64 * core, np.arange(64)] = 1.0
        m["selT"] = bf(chunkP(sel))                     # [128, 4, 64]
        # bucket-sorted sibling tiles
        s0 = core * per_core
        hh = sh_i[s0:s0 + per_core]
        mm_ = sm_i[s0:s0 + per_core]
        ss_ = ss_i[s0:s0 + per_core]
        bucket = (hh // 128) * 16 + (mm_ // 128) * 4 + (ss_ // 128)
        order = np.argsort(bucket, kind="stable")
        counts = np.bincount(bucket, minlength=N_BUCKET)
        if counts.max() > TILES_PER_BUCKET * 128:
            raise RuntimeError(f"sib bucket overflow: {counts.max()}")
        idx_rows = np.zeros((N_SIB_TILE, 3, 128), np.int64)
        slotmap = np.full(N_SIB_TILE * 128, -1, np.int64)
        pos = 0
        for b in range(N_BUCKET):
            n = counts[b]
            sel_idx = order[pos:pos + n]
            pos += n
            t0 = b * TILES_PER_BUCKET
            slot = np.arange(n)
            tt = t0 + slot // 128
            pp = slot % 128
            idx_rows[tt, 0, pp] = hh[sel_idx] - 128 * (b >> 4)
            idx_rows[tt, 1, pp] = mm_[sel_idx] - 128 * ((b >> 2) & 3)
            idx_rows[tt, 2, pp] = ss_[sel_idx] - 128 * (b & 3)
            slotmap[tt * 128 + pp] = s0 + sel_idx
        # host-built one-hots: oh[adj, t*384 + r*128 + p] = 1
        import ml_dtypes as _ml
        oh = np.zeros((128, N_SIB_TILE * 384), _ml.bfloat16)
        cols = np.arange(N_SIB_TILE * 384)
        oh[idx_rows.reshape(-1), cols] = 1.0
        m["sib_oh"] = oh
        in_maps.append(m)
        slotmaps.append(slotmap)
    return in_maps, slotmaps


LAST_EXEC_NS = None


def kernel(**inputs):
    global LAST_EXEC_NS
    _install_ntff_hook()
    from concourse.bass_utils import run_bass_kernel_spmd

    nc = _get_program()
    in_maps, slotmaps = _host_prepare(inputs)
    import os

    trace = os.environ.get("KERNEL_TRACE", "0") == "1"
    res = run_bass_kernel_spmd(nc, in_maps, list(range(NC)), trace=trace)
    LAST_EXEC_NS = res.exec_time_ns
    _CACHE["res"] = res

    ah = np.asarray(inputs["arc_head"]).astype(np.int64)
    am = np.asarray(inputs["arc_mod"]).astype(np.int64)
    Smat = np.zeros((S, S), np.float32)
    sib_scores = np.zeros(ASIB, np.float32)
    for core in range(NC):
        r = res.results[core]
        Smat[core * N_ARC_H:(core + 1) * N_ARC_H] = np.asarray(r["arc_out"])
        sib_flat = np.asarray(r["sib_out"]).T.reshape(-1)  # [tile*128]
        sm = slotmaps[core]
        valid = sm >= 0
        sib_scores[sm[valid]] = sib_flat[valid]
    arc_scores = Smat[ah, am]
    return np.concatenate([arc_scores, sib_scores])


# revision 35
# speedup vs baseline: 1.3001x; 1.1155x over previous
"""Trainium2 Bass kernel for nn_DependencyNeuralModel (dependency parser scorer).

Device strategy (8 NeuronCores, SPMD):
  - Encoder (2-layer BiLSTM over S=512) replicated on every core,
    chunk-parallel: 64 chunks x 2 dirs = 128 batch rows advance in
    lock-step; each chunk warms up from zero over K=16 positions
    (forget-gate contraction makes the warmup error ~6e-3).
    Per step the wx term is injected into PSUM by an identity matmul and
    the gate nonlinearities read PSUM directly, one 512-wide gate region
    at a time, so ACT overlaps the PE stream.
  - Arc scores: A = S^2 exactly, so the full score matrix
    Score[h,m] = w . tanh(headsT[:,h] + modsT[:,m] + DT[:, m-h]) is
    computed densely with NO gather: partition dim = hidden (4 chunks),
    mods as a resident tile, heads column as the ACT bias, and the
    distance embedding as a sliding window into an offset table.
    h rows are sharded across cores (64 per core); the host does the
    final Score[arc_head, arc_mod] scalar fancy-index.
  - Sibling scores: parts bucket-sorted on host by
    (head//128, mod//128, sib//128) into 64 buckets x 3 static tiles;
    each tile needs just 3 one-hot gather matmuls (one 128-row chunk per
    role).  tanh on ACT, fused multiply+reduce on DVE.
Host does index/layout preparation, the final arc fancy-index and
sibling unpermute.
"""
import sys
import types

import numpy as np

sys.path.insert(0, "/opt/trn_rl_repo")

import concourse.bass as bass
import concourse.mybir as mybir
from concourse.tile import TileContext
from concourse.masks import make_identity

S = 512
H = 512
A = 262144
ASIB = 131072
NB = 17
L = 8
K_WARM = 14
NSTEP = K_WARM + L  # 22
NC = 8
F32 = mybir.dt.float32
BF16 = mybir.dt.bfloat16
BINS = np.array(list(range(10)) + list(range(10, 40, 5)) + [40], dtype=np.int64)

N_BUCKET = 64          # (head//128, mod//128, sib//128)
TILES_PER_BUCKET = 3   # 384 slots per bucket; max observed occupancy ~306
N_SIB_TILE = N_BUCKET * TILES_PER_BUCKET  # 192
N_ARC_H = S // NC      # 64 dense score-matrix rows per core
# steps where some chunk hits pos == -1 (8c + s == K_WARM - 1): reset state
_msteps = sorted(K_WARM - 1 - 8 * c for c in range((K_WARM + 7) // 8))
MASK_STEPS = {s: i for i, s in enumerate(_msteps)}


def _install_ntff_hook():
    if "antenv.axon_hooks" in sys.modules:
        return
    mod = types.ModuleType("antenv.axon_hooks")
    state = {"hook": None, "tried": False}

    def set_axon_ntff_profile_hook(hook):
        state["hook"] = hook

    def get_axon_ntff_profile_hook():
        if state["hook"] is None and not state["tried"]:
            state["tried"] = True
            try:
                from trn_agent_boot.trn_boot import _ntff_profile_via_ctypes

                state["hook"] = _ntff_profile_via_ctypes("/opt/axon/libaxon_pjrt.so")
            except Exception:
                state["hook"] = None
        return state["hook"]

    mod.set_axon_ntff_profile_hook = set_axon_ntff_profile_hook
    mod.get_axon_ntff_profile_hook = get_axon_ntff_profile_hook
    import antenv

    antenv.axon_hooks = mod
    sys.modules["antenv.axon_hooks"] = mod


def _legalize_waits(nc):
    """This walrus accepts at most one semaphore wait per instruction;
    split extra waits onto same-engine NOPs placed just before."""
    ctr = [0]
    for f in nc.m.functions:
        for blk in f.blocks:
            out = []
            dirty = False
            for ins in blk.instructions:
                si = ins.sync_info
                if si is not None and si.on_wait and len(si.on_wait) > 1:
                    waits = list(si.on_wait)
                    for w in waits[:-1]:
                        ctr[0] += 1
                        nop = mybir.InstNoOp(name=f"waitfix-{ctr[0]}")
                        nop.engine = ins.engine
                        nop.sync_info = mybir.SyncInfo(on_wait=[w], on_update=[])
                        out.append(nop)
                    ins.sync_info = mybir.SyncInfo(
                        on_wait=[waits[-1]],
                        on_update=list(si.on_update) if si.on_update else [],
                    )
                    dirty = True
                out.append(ins)
            if dirty:
                blk.instructions = out
    return nc


def _lstm_layer(nc, tc, ident, mask_sb, whh_sb, wx_dram, f_dram, b_dram):
    """One BiLSTM layer, chunk-parallel.  128 batch rows: partitions 0:64
    dir0 chunks, 64:128 dir1.  Gate regions (512 cols each) get their own
    PSUM tile; wx is injected by an identity matmul so ACT reads PSUM."""
    import contextlib

    SIG = mybir.ActivationFunctionType.Sigmoid
    TANH = mybir.ActivationFunctionType.Tanh

    with contextlib.ExitStack() as ctx:
        wxp = ctx.enter_context(tc.tile_pool(name="lstm_wx", bufs=2))
        pg = ctx.enter_context(tc.tile_pool(name="lstm_pg", bufs=6, space="PSUM"))
        gp = ctx.enter_context(tc.tile_pool(name="lstm_g", bufs=6))
        cp = ctx.enter_context(tc.tile_pool(name="lstm_c", bufs=8))
        pst = ctx.enter_context(tc.tile_pool(name="lstm_pst", bufs=2, space="PSUM"))
        st = ctx.enter_context(tc.tile_pool(name="lstm_state", bufs=1))

        h_t = st.tile([128, 4, 128], BF16)  # h transposed: [k-part, kc, b]
        c_st = st.tile([128, 512], F32)
        nc.vector.memset(h_t.rearrange("p a b -> p (a b)"), 0.0)
        nc.vector.memset(c_st[:], 0.0)

        for s in range(NSTEP):
            wx = wxp.tile([128, 2048], BF16, tag="wx")
            for d in range(2):
                nc.sync.dma_start(
                    wx[d * 64:(d + 1) * 64, :], wx_dram[d, s:s + 505:8, :]
                )
            # all four wx-injection matmuls first: they depend only on wx, so
            # they fill the PE gap while the previous step's tail completes
            P = {}
            for ng in (1, 0, 2, 3):  # f, i, g, o
                Pt = pg.tile([128, 512], F32, tag="P")
                P[ng] = Pt
                nc.tensor.matmul(
                    Pt[:], lhsT=ident[:], rhs=wx[:, ng * 512:(ng + 1) * 512],
                    start=True, stop=False, skip_group_check=True,
                )
            gate = {}
            for ng in (1, 0, 2, 3):
                for d in range(2):
                    bs = slice(d * 64, (d + 1) * 64)
                    for kc in range(4):
                        nc.tensor.matmul(
                            P[ng][bs, :],
                            lhsT=h_t[:, kc, bs],
                            rhs=whh_sb[:, kc, d, ng * 512:(ng + 1) * 512],
                            start=False,
                            stop=(d == 1 and kc == 3),
                            skip_group_check=True,
                        )
                g = gp.tile([128, 512], BF16, tag=f"g{ng}")
                nc.scalar.activation(g[:], P[ng][:], TANH if ng == 2 else SIG)
                gate[ng] = g
            t1 = cp.tile([128, 512], F32, tag="t1")
            nc.vector.tensor_mul(t1[:], gate[1][:], c_st[:])
            t2 = cp.tile([128, 512], F32, tag="t2")
            nc.vector.tensor_mul(t2[:], gate[0][:], gate[2][:])
            nc.vector.tensor_add(c_st[:], t1[:], t2[:])
            # tail in halves so transposes/copies overlap the second tanh
            h_new = cp.tile([128, 512], BF16, tag="h")
            mi = MASK_STEPS.get(s)
            for half in range(2):
                sl = slice(half * 256, (half + 1) * 256)
                tch = cp.tile([128, 256], BF16, tag=f"tch{half}")
                nc.scalar.activation(tch[:], c_st[:, sl], TANH)
                nc.vector.tensor_mul(h_new[:, sl], gate[3][:, sl], tch[:])
                if mi is not None:
                    nc.vector.tensor_scalar_mul(h_new[:, sl], h_new[:, sl],
                                                mask_sb[:, mi:mi + 1])
                for kc in (2 * half, 2 * half + 1):
                    tp = pst.tile([128, 128], BF16, tag="tr")
                    nc.tensor.transpose(tp[:], h_new[:, kc * 128:(kc + 1) * 128],
                                        ident[:])
                    nc.vector.tensor_copy(h_t[:, kc, :], tp[:])
            if mi is not None:
                nc.vector.tensor_scalar_mul(c_st[:], c_st[:], mask_sb[:, mi:mi + 1])
            if s >= K_WARM:
                o = s - K_WARM
                nc.sync.dma_start(f_dram[o:505 + o:8, :], h_new[0:64, :])
                nc.sync.dma_start(b_dram[o:505 + o:8, :], h_new[64:128, :])


def _transpose_pair(nc, tc, ident, rev, f_dram, b_dram, dstT, dstTrev, one_row):
    """Build [feat, pos] lhsT chunks (and optionally pos-reversed copy) from
    the per-direction output buffers.  dstT/dstTrev: [128, 9, 512] tiles;
    chunk 8 row 0 is set to ones (bias); rest of chunk 8 zero."""
    import contextlib

    with contextlib.ExitStack() as ctx:
        sb = ctx.enter_context(tc.tile_pool(name="tp_sb", bufs=3))
        ps = ctx.enter_context(tc.tile_pool(name="tp_ps", bufs=2, space="PSUM"))
        for dst in (dstT, dstTrev):
            if dst is None:
                continue
            nc.vector.memset(dst[:, 8, :], 0.0)
            nc.vector.tensor_copy(dst[0:1, 8, :], one_row[:])
        for pc in range(4):
            fsrc = sb.tile([128, 512], BF16, tag="fsrc")
            nc.sync.dma_start(fsrc[:], f_dram[pc * 128:(pc + 1) * 128, :])
            bsrc = sb.tile([128, 512], BF16, tag="bsrc")
            nc.sync.dma_start(bsrc[:], b_dram[pc * 128:(pc + 1) * 128, :])
            for j in range(4):
                fs = fsrc[:, j * 128:(j + 1) * 128]
                bs = bsrc[:, j * 128:(j + 1) * 128]
                tp = ps.tile([128, 128], BF16, tag="tp")
                nc.tensor.transpose(tp[:], fs, ident[:])
                nc.vector.tensor_copy(dstT[:, j, pc * 128:(pc + 1) * 128], tp[:])
                if dstTrev is not None:
                    tpr = ps.tile([128, 128], BF16, tag="tpr")
                    nc.tensor.transpose(tpr[:], fs, rev[:])
                    nc.vector.tensor_copy(
                        dstTrev[:, j, (3 - pc) * 128:(4 - pc) * 128], tpr[:])
                # b rows are scan order q; position = 511-q: reverse via rev
                tpb = ps.tile([128, 128], BF16, tag="tpb")
                nc.tensor.transpose(tpb[:], bs, rev[:])
                nc.vector.tensor_copy(
                    dstT[:, 4 + j, (3 - pc) * 128:(4 - pc) * 128], tpb[:])
                if dstTrev is not None:
                    tpb2 = ps.tile([128, 128], BF16, tag="tpb2")
                    nc.tensor.transpose(tpb2[:], bs, ident[:])
                    nc.vector.tensor_copy(
                        dstTrev[:, 4 + j, pc * 128:(pc + 1) * 128], tpb2[:])


def _input_gemm(nc, tc, lhsT_tiles, w_sb, wx_dram, nk):
    """WX[d] = lhsT_d.T @ w[d] -> wx_dram[d, 16:528, :], bf16.
    lhsT_tiles: per-dir [128, nk, 512] SBUF ([feat-part, chunk, pos]).
    w_sb: [128, nk, 2, 2048] SBUF weights."""
    import contextlib

    with contextlib.ExitStack() as ctx:
        sb = ctx.enter_context(tc.tile_pool(name="ig_sb", bufs=4))
        ps = ctx.enter_context(tc.tile_pool(name="ig_ps", bufs=4, space="PSUM"))
        for d in range(2):
            lhsT = lhsT_tiles[d]
            for mc in range(4):
                for ngc in range(4):
                    acc = ps.tile([128, 512], F32, tag="acc")
                    for kc in range(nk):
                        nc.tensor.matmul(
                            acc[:],
                            lhsT=lhsT[:, kc, mc * 128:(mc + 1) * 128],
                            rhs=w_sb[:, kc, d, ngc * 512:(ngc + 1) * 512],
                            start=(kc == 0),
                            stop=(kc == nk - 1),
                        )
                    osb = sb.tile([128, 512], BF16, tag="osb")
                    if (mc + ngc) % 2 == 0:
                        nc.vector.tensor_copy(osb[:], acc[:])
                    else:
                        nc.scalar.copy(osb[:], acc[:])
                    nc.sync.dma_start(
                        wx_dram[d, K_WARM + mc * 128:K_WARM + (mc + 1) * 128,
                                ngc * 512:(ngc + 1) * 512],
                        osb[:],
                    )


def _build(nc):
    TANH = mybir.ActivationFunctionType.Tanh
    embT_f = nc.dram_tensor("embT_f", [128, 3, 512], BF16, kind="ExternalInput")
    embT_b = nc.dram_tensor("embT_b", [128, 3, 512], BF16, kind="ExternalInput")
    wih0T = nc.dram_tensor("wih0T", [128, 3, 2, 2048], BF16, kind="ExternalInput")
    whh0T = nc.dram_tensor("whh0T", [128, 4, 2, 2048], BF16, kind="ExternalInput")
    wih1T = nc.dram_tensor("wih1T", [128, 9, 2, 2048], BF16, kind="ExternalInput")
    whh1T = nc.dram_tensor("whh1T", [128, 4, 2, 2048], BF16, kind="ExternalInput")
    projT = nc.dram_tensor("projT", [128, 9, 2560], BF16, kind="ExternalInput")
    dtexp = nc.dram_tensor("dtexp", [128, 4, 1023], BF16, kind="ExternalInput")
    selT = nc.dram_tensor("selT", [128, 4, 64], BF16, kind="ExternalInput")
    wT_in = nc.dram_tensor("wT_in", [128, 4], BF16, kind="ExternalInput")
    wrep_in = nc.dram_tensor("wrep_in", [128, 3, 512], BF16, kind="ExternalInput")
    sib_oh = nc.dram_tensor("sib_oh", [128, N_SIB_TILE * 384], BF16,
                            kind="ExternalInput")
    mask_in = nc.dram_tensor("mask_in", [128, 2], F32, kind="ExternalInput")
    rev_in = nc.dram_tensor("rev_in", [128, 128], BF16, kind="ExternalInput")
    arc_out = nc.dram_tensor("arc_out", [N_ARC_H, 512], F32, kind="ExternalOutput")
    sib_out = nc.dram_tensor("sib_out", [128, N_SIB_TILE], F32,
                             kind="ExternalOutput")

    wx0 = nc.dram_tensor("wx0", [2, 528, 2048], BF16)
    wx1 = nc.dram_tensor("wx1", [2, 528, 2048], BF16)
    f0d = nc.dram_tensor("f0d", [512, 512], BF16)
    b0d = nc.dram_tensor("b0d", [512, 512], BF16)
    f1d = nc.dram_tensor("f1d", [512, 512], BF16)
    b1d = nc.dram_tensor("b1d", [512, 512], BF16)

    import contextlib

    with TileContext(nc) as tc:
        with contextlib.ExitStack() as ctx:
            const = ctx.enter_context(tc.tile_pool(name="const", bufs=1))
            enc = ctx.enter_context(tc.tile_pool(name="enc", bufs=1))
            sco = ctx.enter_context(tc.tile_pool(name="sco", bufs=1))

            ident = const.tile([128, 128], BF16)
            make_identity(nc, ident[:])
            rev = const.tile([128, 128], BF16)
            nc.sync.dma_start(rev[:], rev_in[:])
            mask_sb = const.tile([128, 2], F32)
            nc.sync.dma_start(mask_sb[:], mask_in[:])
            wT_sb = const.tile([128, 4], BF16)
            nc.sync.dma_start(wT_sb[:], wT_in[:])
            wrep = const.tile([128, 3, 512], BF16)
            nc.sync.dma_start(wrep.rearrange("p a b -> p (a b)"),
                              wrep_in.rearrange("p a b -> p (a b)"))
            sel_sb = const.tile([128, 4, 64], BF16)
            nc.sync.dma_start(sel_sb.rearrange("p a b -> p (a b)"),
                              selT.rearrange("p a b -> p (a b)"))
            one_row = const.tile([1, 512], BF16)
            nc.vector.memset(one_row[:], 1.0)

            # zero-pad warmup rows of WX buffers
            with tc.tile_pool(name="zp", bufs=1) as zp:
                zrow = zp.tile([K_WARM, 2048], BF16)
                nc.vector.memset(zrow[:], 0.0)
                for wxd in (wx0, wx1):
                    for d in range(2):
                        nc.sync.dma_start(wxd[d, 0:K_WARM, :], zrow[:])

            # ---- WX0 ----
            with tc.tile_pool(name="w0", bufs=1) as w0p:
                wih0_sb = w0p.tile([128, 3, 2, 2048], BF16)
                nc.sync.dma_start(wih0_sb.rearrange("p a b c -> p (a b c)"),
                                  wih0T.rearrange("p a b c -> p (a b c)"))
                ef = w0p.tile([128, 3, 512], BF16)
                nc.sync.dma_start(ef.rearrange("p a b -> p (a b)"),
                                  embT_f.rearrange("p a b -> p (a b)"))
                eb = w0p.tile([128, 3, 512], BF16)
                nc.sync.dma_start(eb.rearrange("p a b -> p (a b)"),
                                  embT_b.rearrange("p a b -> p (a b)"))
                _input_gemm(nc, tc, [ef, eb], wih0_sb, wx0, 3)

                # ---- layer 0 (whh0 shares this scope's lifetime) ----
                whh0_sb = w0p.tile([128, 4, 2, 2048], BF16)
                nc.sync.dma_start(whh0_sb.rearrange("p a b c -> p (a b c)"),
                                  whh0T.rearrange("p a b c -> p (a b c)"))
                _lstm_layer(nc, tc, ident, mask_sb, whh0_sb, wx0, f0d, b0d)

            # ---- x1T / x1Trev ----
            x1T = enc.tile([128, 9, 512], BF16, tag="x1T")
            x1Trev = enc.tile([128, 9, 512], BF16, tag="x1Trev")
            _transpose_pair(nc, tc, ident, rev, f0d, b0d, x1T, x1Trev, one_row)

            # ---- WX1 + layer 1 ----
            with tc.tile_pool(name="w1", bufs=1) as w1p:
                wih1_sb = w1p.tile([128, 9, 2, 2048], BF16)
                nc.sync.dma_start(wih1_sb.rearrange("p a b c -> p (a b c)"),
                                  wih1T.rearrange("p a b c -> p (a b c)"))
                _input_gemm(nc, tc, [x1T, x1Trev], wih1_sb, wx1, 9)
            with tc.tile_pool(name="w1b", bufs=1) as w1bp:
                whh1_sb = w1bp.tile([128, 4, 2, 2048], BF16)
                nc.sync.dma_start(whh1_sb.rearrange("p a b c -> p (a b c)"),
                                  whh1T.rearrange("p a b c -> p (a b c)"))
                _lstm_layer(nc, tc, ident, mask_sb, whh1_sb, wx1, f1d, b1d)

            # ---- statesT ----
            stT = enc.tile([128, 9, 512], BF16, tag="x1T")  # reuse slot
            _transpose_pair(nc, tc, ident, rev, f1d, b1d, stT, None, one_row)

            # ---- projection tables ----
            tables_sb = sco.tile([128, 4, 1536], BF16, tag="tables")
            heads_pos = sco.tile([128, 4, 512], BF16, tag="heads_pos")
            modsT = sco.tile([128, 4, 512], BF16, tag="modsT")
            headsb = sco.tile([128, 4, 64], F32, tag="headsb")
            with contextlib.ExitStack() as c2:
                pj = c2.enter_context(tc.tile_pool(name="pj", bufs=1))
                ps2 = c2.enter_context(tc.tile_pool(name="tb_ps", bufs=5,
                                                    space="PSUM"))
                ps2b = c2.enter_context(tc.tile_pool(name="tb_ps2", bufs=1,
                                                     space="PSUM"))
                ps2c = c2.enter_context(tc.tile_pool(name="tb_ps3", bufs=2,
                                                     space="PSUM"))
                projT_sb = pj.tile([128, 9, 2560], BF16)
                nc.sync.dma_start(projT_sb.rearrange("p a b -> p (a b)"),
                                  projT.rearrange("p a b -> p (a b)"))
                # sib tables + heads in pos-part layout
                for mc in range(4):
                    for r in range(4):  # 0..2 sib tables, 3 = heads
                        toff = (2 + r) * 512 if r < 3 else 0
                        acc = ps2.tile([128, 512], F32, tag="acc")
                        for kc in range(9):
                            nc.tensor.matmul(
                                acc[:],
                                lhsT=stT[:, kc, mc * 128:(mc + 1) * 128],
                                rhs=projT_sb[:, kc, toff:toff + 512],
                                start=(kc == 0), stop=(kc == 8),
                            )
                        if r < 3:
                            nc.vector.tensor_copy(
                                tables_sb[:, mc, r * 512:(r + 1) * 512], acc[:])
                        else:
                            nc.scalar.copy(heads_pos[:, mc, :], acc[:])
                # mods in hidden-part layout
                for hc in range(4):
                    acc = ps2.tile([128, 512], F32, tag="acc")
                    for kc in range(9):
                        nc.tensor.matmul(
                            acc[:],
                            lhsT=projT_sb[:, kc, 512 + hc * 128:512 + hc * 128 + 128],
                            rhs=stT[:, kc, :],
                            start=(kc == 0), stop=(kc == 8),
                        )
                    nc.vector.tensor_copy(modsT[:, hc, :], acc[:])
                # per-core heads columns: select 64 pos, then transpose
                hsel_ps = ps2b.tile([64, 512], F32, tag="hsel")
                for mc in range(4):
                    nc.tensor.matmul(
                        hsel_ps[:], lhsT=sel_sb[:, mc, :], rhs=heads_pos[:, mc, :],
                        start=(mc == 0), stop=(mc == 3),
                    )
                hsel_sb = pj.tile([64, 512], BF16)
                nc.scalar.copy(hsel_sb[:], hsel_ps[:])
                for hc in range(4):
                    tp = ps2c.tile([128, 64], BF16, tag="htp")
                    nc.tensor.transpose(tp[:], hsel_sb[:, hc * 128:(hc + 1) * 128],
                                        ident[0:64, 0:64])
                    nc.vector.tensor_copy(headsb[:, hc, :], tp[:])

            dtexp_sb = sco.tile([128, 4, 1023], BF16, tag="dtexp")
            nc.sync.dma_start(dtexp_sb.rearrange("p a b -> p (a b)"),
                              dtexp.rearrange("p a b -> p (a b)"))
            sib_sb = sco.tile([128, N_SIB_TILE], F32, tag="sib_sb")

            # ---- scoring: interleave sib tiles (PE/DVE) with arc rows (ACT) --
            with contextlib.ExitStack() as c3:
                ohp = c3.enter_context(tc.tile_pool(name="sc_oh", bufs=4))
                sap = c3.enter_context(tc.tile_pool(name="sc_sa", bufs=2,
                                                    space="PSUM"))
                thp = c3.enter_context(tc.tile_pool(name="sc_th", bufs=3))
                arp = c3.enter_context(tc.tile_pool(name="sc_ar", bufs=4))
                rwp = c3.enter_context(tc.tile_pool(name="sc_rw", bufs=2,
                                                    space="PSUM"))

                for u in range(N_SIB_TILE // 3):
                    # ---------- three sibling tiles, batched dot ----------
                    th3 = thp.tile([128, 3, 512], BF16, tag="th3")
                    for i in range(3):
                        t = u * 3 + i
                        bucket = t // TILES_PER_BUCKET
                        ccs = (bucket >> 4, (bucket >> 2) & 3, bucket & 3)
                        oh = ohp.tile([128, 384], BF16, tag="oh")
                        nc.sync.dma_start(oh[:], sib_oh[:, t * 384:(t + 1) * 384])
                        acc = sap.tile([128, 512], F32, tag="acc")
                        for r in range(3):
                            nc.tensor.matmul(
                                acc[:],
                                lhsT=oh[:, r * 128:(r + 1) * 128],
                                rhs=tables_sb[:, ccs[r], r * 512:(r + 1) * 512],
                                start=(r == 0), stop=(r == 2),
                            )
                        nc.scalar.activation(th3[:, i, :], acc[:], TANH)
                    scr3 = thp.tile([128, 3, 512], BF16, tag="scr3")
                    nc.vector.tensor_mul(
                        scr3.rearrange("p a b -> p (a b)"),
                        th3.rearrange("p a b -> p (a b)"),
                        wrep.rearrange("p a b -> p (a b)"),
                    )
                    nc.vector.tensor_reduce(
                        sib_sb[:, u * 3:u * 3 + 3], scr3[:],
                        mybir.AxisListType.X, mybir.AluOpType.add,
                    )
                    # ---------- dense arc row ----------
                    if True:
                        h = u
                        marg = arp.tile([128, 4, 512], BF16, tag="marg")
                        nc.gpsimd.tensor_add(
                            marg.rearrange("p a b -> p (a b)"),
                            modsT.rearrange("p a b -> p (a b)"),
                            dtexp_sb[:, :, 511 - h:1023 - h],
                        )
                        tha = arp.tile([128, 4, 512], BF16, tag="tha")
                        for hc in range(4):
                            nc.scalar.activation(
                                tha[:, hc, :], marg[:, hc, :], TANH,
                                bias=headsb[:, hc, h:h + 1],
                            )
                        row_ps = rwp.tile([1, 512], F32, tag="row")
                        for hc in range(4):
                            nc.tensor.matmul(
                                row_ps[:],
                                lhsT=wT_sb[:, hc:hc + 1],
                                rhs=tha[:, hc, :],
                                start=(hc == 0), stop=(hc == 3),
                            )
                        row_sb = arp.tile([1, 512], F32, tag="rowsb")
                        if h % 2 == 0:
                            nc.vector.tensor_copy(row_sb[:], row_ps[:])
                        else:
                            nc.scalar.copy(row_sb[:], row_ps[:])
                        nc.sync.dma_start(arc_out[h:h + 1, :], row_sb[:])

                nc.sync.dma_start(sib_out[:], sib_sb[:])
    return nc


_CACHE = {}


def _get_program():
    if "nc" not in _CACHE:
        nc = bass.Bass()
        _build(nc)
        _legalize_waits(nc)
        _CACHE["nc"] = nc
    return _CACHE["nc"]


def _host_prepare(inputs):
    import ml_dtypes

    f32 = np.float32

    def bf(x):
        return np.asarray(x, f32).astype(ml_dtypes.bfloat16)

    words = np.asarray(inputs["words"]).astype(np.int64)
    tags = np.asarray(inputs["tags"]).astype(np.int64)
    word_emb = np.asarray(inputs["word_emb"], f32)
    tag_emb = np.asarray(inputs["tag_emb"], f32)
    emb = np.concatenate([word_emb[words], tag_emb[tags]], axis=-1)  # [512, 364]
    emb_aug = np.concatenate([emb, np.ones((S, 1), f32)], axis=1)    # [512, 365]

    def packT(x, rows):  # -> [rows(pad), S] = x.T zero-padded
        out = np.zeros((rows, x.shape[0]), f32)
        out[: x.shape[1]] = x.T
        return out

    def chunkP(x):  # [K*128, ...] -> [128, K, ...]
        sh = x.shape
        return x.reshape(sh[0] // 128, 128, *sh[1:]).transpose(
            1, 0, *range(2, x.ndim + 1)).copy()

    embT_f = bf(chunkP(packT(emb_aug, 384)))
    embT_b = bf(chunkP(packT(emb_aug[::-1], 384)))

    def wih_pack(Wih, bih, bhh, kdim, rows):
        out = np.zeros((rows, 2, 4 * H), f32)
        for d in range(2):
            out[:kdim, d] = np.asarray(Wih[d], f32).T
            out[kdim, d] = np.asarray(bih[d], f32) + np.asarray(bhh[d], f32)
        return out

    wih0T = bf(chunkP(wih_pack(inputs["Wih0"], inputs["bih0"], inputs["bhh0"],
                               364, 384)))
    wih1T = bf(chunkP(wih_pack(inputs["Wih1"], inputs["bih1"], inputs["bhh1"],
                               1024, 1152)))

    def whh_pack(Whh):
        out = np.zeros((128, 4, 2, 4 * H), f32)
        for d in range(2):
            wt = np.asarray(Whh[d], f32).T  # [512 k, 2048 g]
            out[:, :, d, :] = wt.reshape(4, 128, 4 * H).transpose(1, 0, 2)
        return out

    whh0T = bf(whh_pack(inputs["Whh0"]))
    whh1T = bf(whh_pack(inputs["Whh1"]))

    projs = [inputs["head_W"], inputs["mod_W"], inputs["sib_head_W"],
             inputs["sib_mod_W"], inputs["sib_sib_W"]]
    projT = np.zeros((1152, 5 * H), f32)
    for i, W in enumerate(projs):
        projT[:1024, i * H:(i + 1) * H] = np.asarray(W, f32).T
    projT = bf(chunkP(projT))

    # distance table expanded over offsets delta = m - h in [-511, 511]
    D = (np.asarray(inputs["dist_emb"], f32) @ np.asarray(inputs["dist_W"], f32).T
         + np.asarray(inputs["dist_b"], f32))          # [34, H]
    delta = np.arange(-511, 512)
    bidx = np.searchsorted(BINS, np.abs(delta), side="right") - 1
    didx = np.where(delta > 0, bidx, bidx + NB)
    DTg = D[didx]                                       # [1023, H]

    w = np.asarray(inputs["arc_w"], f32).reshape(512)
    wT = bf(w.reshape(4, 128).T)                        # [128, 4]
    wrep = bf(np.tile(w[None, None, :], (128, 3, 1)))   # [128, 3, 512]

    mask = np.zeros((128, len(MASK_STEPS)), f32)
    for s, mi in MASK_STEPS.items():
        c = np.arange(64)
        v = ((8 * c + s) > (K_WARM - 1)).astype(f32)
        mask[0:64, mi] = v
        mask[64:128, mi] = v
    revm = np.zeros((128, 128), f32)
    revm[np.arange(128), 127 - np.arange(128)] = 1.0

    base = {
        "embT_f": embT_f, "embT_b": embT_b,
        "wih0T": wih0T, "whh0T": whh0T, "wih1T": wih1T, "whh1T": whh1T,
        "projT": projT, "wT_in": wT, "wrep_in": wrep,
        "mask_in": mask, "rev_in": bf(revm),
    }

    sh_i = np.asarray(inputs["sib_head"]).astype(np.int64)
    sm_i = np.asarray(inputs["sib_mod"]).astype(np.int64)
    ss_i = np.asarray(inputs["sib_sib"]).astype(np.int64)
    per_core = ASIB // NC

    in_maps = []
    slotmaps = []
    for core in range(NC):
        m = dict(base)
        # shifted distance window: dtexp_core[j] = DTg[j - 64*core]
        dte = np.zeros((1023, 512), f32)
        lo = 64 * core
        dte[lo:] = DTg[: 1023 - lo]
        m["dtexp"] = bf(chunkP(dte.T.copy()))           # [128, 4, 1023]
        # heads-column selector: one-hot of position 64*core + j
        sel = np.zeros((512, 64), f32)
        sel[np.arange(64) + 64 * core, np.arange(64)] = 1.0
        m["selT"] = bf(chunkP(sel))                     # [128, 4, 64]
        # bucket-sorted sibling tiles
        s0 = core * per_core
        hh = sh_i[s0:s0 + per_core]
        mm_ = sm_i[s0:s0 + per_core]
        ss_ = ss_i[s0:s0 + per_core]
        bucket = (hh // 128) * 16 + (mm_ // 128) * 4 + (ss_ // 128)
        order = np.argsort(bucket, kind="stable")
        counts = np.bincount(bucket, minlength=N_BUCKET)
        if counts.max() > TILES_PER_BUCKET * 128:
            raise RuntimeError(f"sib bucket overflow: {counts.max()}")
        idx_rows = np.zeros((N_SIB_TILE, 3, 128), np.int64)
        slotmap = np.full(N_SIB_TILE * 128, -1, np.int64)
        pos = 0
        for b in range(N_BUCKET):
            n = counts[b]
            sel_idx = order[pos:pos + n]
            pos += n
            t0 = b * TILES_PER_BUCKET
            slot = np.arange(n)
            tt = t0 + slot // 128
            pp = slot % 128
            idx_rows[tt, 0, pp] = hh[sel_idx] - 128 * (b >> 4)
            idx_rows[tt, 1, pp] = mm_[sel_idx] - 128 * ((b >> 2) & 3)
            idx_rows[tt, 2, pp] = ss_[sel_idx] - 128 * (b & 3)
            slotmap[tt * 128 + pp] = s0 + sel_idx
        # host-built one-hots: oh[adj, t*384 + r*128 + p] = 1
        import ml_dtypes as _ml
        oh = np.zeros((128, N_SIB_TILE * 384), _ml.bfloat16)
        cols = np.arange(N_SIB_TILE * 384)
        oh[idx_rows.reshape(-1), cols] = 1.0
        m["sib_oh"] = oh
        in_maps.append(m)
        slotmaps.append(slotmap)
    return in_maps, slotmaps


LAST_EXEC_NS = None


def kernel(**inputs):
    global LAST_EXEC_NS
    _install_ntff_hook()
    from concourse.bass_utils import run_bass_kernel_spmd

    nc = _get_program()
    in_maps, slotmaps = _host_prepare(inputs)
    import os

    trace = os.environ.get("KERNEL_TRACE", "0") == "1"
    res = run_bass_kernel_spmd(nc, in_maps, list(range(NC)), trace=trace)
    LAST_EXEC_NS = res.exec_time_ns
    _CACHE["res"] = res

    ah = np.asarray(inputs["arc_head"]).astype(np.int64)
    am = np.asarray(inputs["arc_mod"]).astype(np.int64)
    Smat = np.zeros((S, S), np.float32)
    sib_scores = np.zeros(ASIB, np.float32)
    for core in range(NC):
        r = res.results[core]
        Smat[core * N_ARC_H:(core + 1) * N_ARC_H] = np.asarray(r["arc_out"])
        sib_flat = np.asarray(r["sib_out"]).T.reshape(-1)  # [tile*128]
        sm = slotmaps[core]
        valid = sm >= 0
        sib_scores[sm[valid]] = sib_flat[valid]
    arc_scores = Smat[ah, am]
    return np.concatenate([arc_scores, sib_scores])
